# revision 43
# baseline (speedup 1.0000x reference)
"""PointNet++-lite segmentation on 8 Trainium2 cores (batch-parallel, one
point cloud per core). Self-contained: hardcodes shapes from the problem spec.

Per-core pipeline (all on device):
  embed MLP -> SA1 (KNN top-32 of 16384, gather, 2-layer MLP, max-pool)
  -> SA2 (KNN top-32 of 1024) -> FP2/FP1 (3-NN inverse-distance interp)
  -> global-max head MLP -> (16384, 13) logits.

Performance design (~2.2x vs the fp32 per-stage baseline in TimelineSim):
- KNN ranking m = 2 q.x - |x|^2 (row-constant |q|^2 dropped) on the PE from
  bf16 host-precomputed coordinate tables (1 cyc/row); top-32 selection on
  the DVE: max8/max_index over 1024-wide PSUM windows, then the global index
  is packed into the low 14 mantissa bits of each candidate value so ONE
  4-round max8/match_replace pass extracts values+indices together (ties
  within 2^-9 relative resolve by index - below the bf16 coord noise).
- All coordinate tables are host inputs, so every KNN selection depends only
  on DMA-loaded constants: the DVE selection stream (the critical resource,
  ~94% busy) is software-pipelined across stages - sel(i+k) is emitted ahead
  of mlp(i) so the in-order engine queues overlap selection with MLP work.
- SA1/SA2 MLPs, gather tables (tab0/1/2/f, f0T) and neighbor tiles in bf16
  (1 cyc/row matmuls + transposes, half DMA); FP/head MLPs in fp32r (TF32,
  1 cyc/row, producers write rounded F32R APs per walrus' requirement).
- GPSIMD only triggers gathers/iota (TRN2 walrus rejects tensor ops there);
  max-pool reduces + interp scalar_tensor_tensor chains stay on DVE; the
  head tail is rebalanced ACT<->DVE and spreads matmuls over 3 PSUM pools.
- Bulk table writes are split across DMA engines so the latency-critical
  wrap_idx index bounces never queue behind a multi-us transfer; groups of
  4-8 PE transposes are packed into one PSUM bank and drained by a single
  wide ACT copy (embed 128->16 copies, FP1 8->2 per group).
"""

from contextlib import ExitStack

import numpy as np

import concourse.bass as bass
import concourse.mybir as mybir
from concourse.bacc import Bacc
from concourse.bass_utils import run_bass_kernel_spmd
from concourse.masks import make_identity
from concourse.tile import TileContext

F32 = mybir.dt.float32
F32R = mybir.dt.float32r
BF16 = mybir.dt.bfloat16
U16 = mybir.dt.uint16
U8 = mybir.dt.uint8
I16 = mybir.dt.int16
U32 = mybir.dt.uint32
AF = mybir.ActivationFunctionType
ALU = mybir.AluOpType
AX = mybir.AxisListType

P = 128
N = 16384
S1, K1 = 1024, 32
S2, K2 = 256, 32
NCLS = 13
NEG = -3.0e38

NT = N // P        # 128 point tiles
T1 = S1 // P       # 8 SA1 query tiles
T2 = S2 // P       # 2 SA2 query tiles
GT = 8             # FP1 group size (query tiles per group)
NG = NT // GT      # 16 FP1 groups
TAB0_W = 128       # bf16 rows: [feat0(64), xyz(3), zero, pad...] 256B
TAB1_W = 256       # bf16 rows: [feat1(128), xyz1(3), pad...] 512B


def build_nc():
    nc = Bacc()

    xT_in = nc.dram_tensor("xT", [6, N], F32, kind="ExternalInput")
    # host-precomputed coordinate tables (bf16; see _per_core_inputs)
    bigT_in = nc.dram_tensor("bigT", [5, N], BF16, kind="ExternalInput")
    q1t_in = nc.dram_tensor("q1t", [5, S1], BF16, kind="ExternalInput")
    c1t_in = nc.dram_tensor("c1t", [5, S1], BF16, kind="ExternalInput")
    r2x_in = nc.dram_tensor("r2x", [5, S1], BF16, kind="ExternalInput")
    q2t_in = nc.dram_tensor("q2t", [5, S2], BF16, kind="ExternalInput")
    rF2_in = nc.dram_tensor("rF2", [5, S2], BF16, kind="ExternalInput")
    nq1_in = nc.dram_tensor("nq1", [3, S1], BF16, kind="ExternalInput")
    nq2_in = nc.dram_tensor("nq2", [3, S2], BF16, kind="ExternalInput")
    sqpm_in = nc.dram_tensor("sqpm", [P, NT], F32, kind="ExternalInput")
    sq1pm_in = nc.dram_tensor("sq1pm", [P, T1], F32, kind="ExternalInput")
    xyzb_in = nc.dram_tensor("xyzb", [P, NT * 4], BF16, kind="ExternalInput")
    x1pm_in = nc.dram_tensor("x1pm", [P, T1 * 3], BF16, kind="ExternalInput")
    # host-assembled bf16 weights for the SA1/SA2 MLPs
    w1aug_in = nc.dram_tensor("w1augb", [68, 128], BF16, kind="ExternalInput")
    w1q_in = nc.dram_tensor("w1qb", [4, 128], BF16, kind="ExternalInput")
    v1A_in = nc.dram_tensor("v1Ab", [128, 256], BF16, kind="ExternalInput")
    v1rel_in = nc.dram_tensor("v1relb", [3, 256], BF16, kind="ExternalInput")
    v1q_in = nc.dram_tensor("v1qb", [4, 256], BF16, kind="ExternalInput")
    v2_in = nc.dram_tensor("v2b", [256, 256], BF16, kind="ExternalInput")
    w2_in = nc.dram_tensor("w2b", [128, 128], BF16, kind="ExternalInput")
    f2w1b_in = nc.dram_tensor("f2w1b", [256, 128], BF16, kind="ExternalInput")
    f1w1a_in = nc.dram_tensor("f1w1ab", [64, 128], BF16, kind="ExternalInput")
    f1w1b_in = nc.dram_tensor("f1w1bb", [128, 128], BF16, kind="ExternalInput")
    hw1a_in = nc.dram_tensor("hw1ab", [128, 128], BF16, kind="ExternalInput")
    wdecl = [
        ("embw", [6, 64]), ("embb", [64, 1]),
        ("b2", [128, 1]), ("c2", [128, 2]),
        ("f2w1", [384, 128]), ("f2b1", [128, 1]), ("f2w2", [128, 128]), ("f2b2", [128, 1]),
        ("f1w1", [192, 128]), ("f1b1", [128, 1]), ("f1w2", [128, 128]), ("f1b2", [128, 1]),
        ("hw1", [256, 128]), ("hb1", [128, 1]), ("hw2", [128, 64]), ("hb2", [64, 1]),
        ("hw3", [64, 13]), ("hb3", [13, 1]),
    ]
    din = {nm: nc.dram_tensor(nm, sh, F32, kind="ExternalInput") for nm, sh in wdecl}
    out_d = nc.dram_tensor("out", [NCLS, N], F32, kind="ExternalOutput")

    tab0_d = nc.dram_tensor("tab0", [N, TAB0_W], BF16)
    tab1_d = nc.dram_tensor("tab1", [S1, TAB1_W], BF16)
    tab2_d = nc.dram_tensor("tab2", [S2, 256], BF16)
    tabf_d = nc.dram_tensor("tabf", [S1, 128], BF16)
    f0T_d = nc.dram_tensor("f0T", [64, N], BF16)
    fuT_d = nc.dram_tensor("fuT", [128, N], BF16)
    ib1_ds = [nc.dram_tensor(f"ib1_{t}", [16, 256], I16) for t in range(T1)]
    ib2_ds = [nc.dram_tensor(f"ib2_{t}", [16, 256], I16) for t in range(T2)]
    ibf2_d = nc.dram_tensor("ibf2", [16, 192], I16)
    ibf1_d = nc.dram_tensor("ibf1", [16, NT * 3 * 8], I16)

    with TileContext(nc) as tc, ExitStack() as ctx:
        cst = ctx.enter_context(tc.tile_pool(name="cst", bufs=1))
        psA = ctx.enter_context(tc.tile_pool(name="psA", bufs=2, space="PSUM"))
        psT = ctx.enter_context(tc.tile_pool(name="psT", bufs=2, space="PSUM"))
        psS = ctx.enter_context(tc.tile_pool(name="psS", bufs=2, space="PSUM"))
        wk = ctx.enter_context(tc.tile_pool(name="wk", bufs=2))

        ident = cst.tile([P, P], F32, tag="ident", name="ident")
        make_identity(nc, ident[:])
        identb = cst.tile([P, P], BF16, tag="identb", name="identb")
        make_identity(nc, identb[:])

        def mmtile():
            return psA.tile([P, 512], F32, tag="mm", name="mm")

        def rr(ap):
            return ap.bitcast(F32R)

        def mmr(out, lhsT, rhs, **kw):
            """fp32r (TF32) matmul: 1 cyc/row vs fp32's 4 for wide outputs.
            Every producer of an fp32r input must write through an F32R-typed
            out AP (walrus requires inputs 'rounded to FP32r')."""
            nc.tensor.matmul(out=out, lhsT=lhsT.bitcast(F32R),
                             rhs=rhs.bitcast(F32R), **kw)

        def transf(in_ap):
            """fp32 PE transpose: in_(p,f) -> psum slice (f,p)."""
            pt = psT.tile([P, 512], F32, tag="trans", name="trans")
            k = in_ap.shape[0]
            f = in_ap.shape[-1]
            nc.tensor.matmul(out=pt[:f, :k], lhsT=in_ap, rhs=ident[:k, :k],
                             is_transpose=True, start=True, stop=True)
            return pt[:f, :k]

        def psb():
            """bf16 view of an F32 psT bank (shares the same 2 banks)."""
            ptf = psT.tile([P, 512], F32, tag="trans", name="trans")
            return ptf[:].bitcast(BF16)[:, 0:512]

        # ---------------- constants / weights ----------------
        def load(name, src, shape, dtype=F32, rnd=False, eng=None):
            t = cst.tile(list(shape), dtype, tag=name, name=name)
            e = eng or nc.sync
            if rnd:
                e.dma_start(out=rr(t[:]), in_=rr(src))
            else:
                e.dma_start(out=t[:], in_=src)
            return t

        # stage01/sel-critical loads first; the bulk (weights, later-stage
        # tables) is emitted after sel(2) so the Sync queue serves the
        # latency-critical wrap/tab0 traffic early.
        bigT = load("bigTsb", bigT_in[:], (5, N), BF16, eng=nc.sync)
        q1t = load("q1tsb", q1t_in[:], (5, S1), BF16, eng=nc.sync)
        xyzb = load("xyzbsb", xyzb_in[:], (P, NT * 4), BF16, eng=nc.sync)
        embw = load("embw", din["embw"][:], (6, 64), rnd=True)
        embb = load("embb", din["embb"][:], (64, 1))

        def late_loads():
            return (
                load("c1tsb", c1t_in[:], (5, S1), BF16, eng=nc.sync),
                load("r2xsb", r2x_in[:], (5, S1), BF16, eng=nc.sync),
                load("q2tsb", q2t_in[:], (5, S2), BF16, eng=nc.sync),
                load("rF2sb", rF2_in[:], (5, S2), BF16, eng=nc.sync),
                load("nq1sb", nq1_in[:], (3, S1), BF16, eng=nc.sync),
                load("nq2sb", nq2_in[:], (3, S2), BF16, eng=nc.sync),
                load("sqpmsb", sqpm_in[:], (P, NT), eng=nc.sync),
                load("sq1pmsb", sq1pm_in[:], (P, T1), eng=nc.sync),
                load("x1pmsb", x1pm_in[:], (P, T1 * 3), BF16, eng=nc.sync),
            )
        def late_weights():
            b2sb = load("b2sb", din["b2"][:], (128, 1))
            c2sb = load("c2sb", din["c2"][:], (128, 2))
            f2b1 = load("f2b1", din["f2b1"][:], (128, 1))
            f2w2 = load("f2w2", din["f2w2"][:], (128, 128), rnd=True)
            f2b2 = load("f2b2", din["f2b2"][:], (128, 1))
            f1b1 = load("f1b1", din["f1b1"][:], (128, 1))
            f1w2 = load("f1w2", din["f1w2"][:], (128, 128), rnd=True)
            f1b2 = load("f1b2", din["f1b2"][:], (128, 1))
            hb1 = load("hb1", din["hb1"][:], (128, 1))
            hw2 = load("hw2", din["hw2"][:], (128, 64), rnd=True)
            hb2 = load("hb2", din["hb2"][:], (64, 1))
            hw3 = load("hw3", din["hw3"][:], (64, 13), rnd=True)
            hb3 = load("hb3", din["hb3"][:], (13, 1))
            f2w1 = [load("f2w10", din["f2w1"][0:128, :], (128, 128), rnd=True)]
            f2w1 += [load(f"f2w1{k}", f2w1b_in[(k - 1) * 128:k * 128, :],
                          (128, 128), BF16) for k in (1, 2)]
            f1w1a = load("f1w1a", f1w1a_in[:], (64, 128), BF16)
            f1w1b = load("f1w1b", f1w1b_in[:], (128, 128), BF16)
            hw1a = load("hw1a", hw1a_in[:], (128, 128), BF16)
            hw1b = load("hw1b", din["hw1"][128:256, :], (128, 128))
            w1aug = load("w1augsb", w1aug_in[:], (68, 128), BF16)
            w1q = load("w1qsb", w1q_in[:], (4, 128), BF16)
            v1A = [load(f"v1A{h}", v1A_in[:, h * 128:(h + 1) * 128],
                        (128, 128), BF16) for h in range(2)]
            v1rel = [load(f"v1rel{h}", v1rel_in[:, h * 128:(h + 1) * 128],
                          (3, 128), BF16) for h in range(2)]
            v1q = [load(f"v1q{h}", v1q_in[:, h * 128:(h + 1) * 128], (4, 128),
                        BF16) for h in range(2)]
            v2sb = [[load(f"v2{k}{h}",
                          v2_in[k * 128:(k + 1) * 128, h * 128:(h + 1) * 128],
                          (128, 128), BF16) for h in range(2)]
                    for k in range(2)]
            w2sb = load("w2sb", w2_in[:], (128, 128), BF16)
            return (b2sb, c2sb, f2b1, f2w2, f2b2, f1b1, f1w2, f1b2, hb1, hw2,
                    hb2, hw3, hb3, f2w1, f1w1a, f1w1b, hw1a, hw1b, w1aug, w1q,
                    v1A, v1rel, v1q, v2sb, w2sb)

        offs = cst.tile([P, 128], U16, tag="offs", name="offs")
        nc.gpsimd.iota(out=offs[:], pattern=[[1024, 16], [0, 8]], base=0,
                       channel_multiplier=0)
        mskhi = cst.tile([P, 1], U32, tag="mskhi", name="mskhi")
        nc.vector.memset(mskhi[:], 0xFFFFC000)
        msklo = cst.tile([P, 1], U32, tag="msklo", name="msklo")
        nc.vector.memset(msklo[:], 0x3FFF)

        def wrap_idx(src_i16, bounce_ap, tag, bufs=2):
            """src (128, M) i16, flat order i = j*128+p -> replicated wrapped
            idx tile (128, 8*M) via DRAM bounce."""
            M = src_i16.shape[-1]
            bw = bounce_ap.rearrange("c (j e) -> c j e", e=8)
            for ph in range(8):
                nc.sync.dma_start(out=bw[:, 0:M, ph],
                                  in_=src_i16[ph * 16:(ph + 1) * 16, :])
            idxt = wk.tile([P, 8 * M], I16, tag=tag, name=tag, bufs=bufs)
            for g in range(8):
                nc.sync.dma_start(out=idxt[g * 16:(g + 1) * 16, :],
                                  in_=bounce_ap)
            return idxt

        # persistent cross-stage tensors
        nqb1, nqb2 = [], []
        for i in range(2):
            t = cst.tile([4, 512], BF16, tag=f"nqb1{i}", name=f"nqb1{i}")
            nc.vector.memset(t[:], 1.0)
            nqb1.append(t)
            t = cst.tile([4, 512], BF16, tag=f"nqb2{i}", name=f"nqb2{i}")
            nc.vector.memset(t[:], 1.0)
            nqb2.append(t)
        feat1T = cst.tile([P, S1], F32, tag="feat1T", name="feat1T")
        feat2T = [cst.tile([P, S2], F32, tag=f"feat2T{h}", name=f"feat2T{h}")
                  for h in range(2)]
        f1upT = cst.tile([P, S1], F32, tag="f1upT", name="f1upT")
        gfacc = cst.tile([P, 1], F32, tag="gfacc", name="gfacc")
        zroP = cst.tile([P, 512], F32, tag="zroP", name="zroP")
        nc.vector.memset(zroP[:], 0.0)
        biasH = cst.tile([P, 1], F32, tag="biasH", name="biasH")

        def interp_weights(sqpm_ap, v8_ap, nt, w_out):
            """d2 = |q|^2 - m -> dist -> normalized inv-dist weights -> w_out"""
            d2 = wk.tile([P, nt, 3], F32, tag="ipd2", name="ipd2")
            nc.vector.tensor_tensor(
                out=d2[:], in0=sqpm_ap.unsqueeze(2).to_broadcast([P, nt, 3]),
                in1=v8_ap, op=ALU.subtract)
            nc.scalar.activation(out=d2[:], in_=d2[:], func=AF.Relu)
            nc.scalar.activation(out=d2[:], in_=d2[:], func=AF.Sqrt)
            nc.vector.tensor_scalar_max(d2[:], d2[:], 1e-10)
            nc.vector.reciprocal(out=w_out, in_=d2[:])
            ws = wk.tile([P, nt], F32, tag="ipws", name="ipws")
            nc.vector.tensor_reduce(out=ws[:], in_=w_out, axis=AX.X, op=ALU.add)
            nc.vector.reciprocal(out=ws[:], in_=ws[:])
            nc.vector.tensor_tensor(
                out=w_out, in0=w_out,
                in1=ws[:].unsqueeze(2).to_broadcast([P, nt, 3]), op=ALU.mult)

        # ---------------- stage emitters ----------------
        def stage01():
            """embed MLP + tab0 build (PE/ACT/DMA; no DVE)."""
            for q4 in range(4):
                ts_ = slice(q4 * (NT // 4), (q4 + 1) * (NT // 4))
                nc.sync.dma_start(
                    out=tab0_d.rearrange("(t p) c -> p t c", p=P)[
                        :, ts_, 64:68],
                    in_=xyzb[:].rearrange("p (t c) -> p t c", c=4)[:, ts_, :])
            for g in range(16):
                stage = wk.tile([P, 8, 64], BF16, tag="tab0stage",
                                name="tab0stage")
                f0cs = []
                for cc in range(2):
                    c = g * 2 + cc
                    xc = wk.tile([6, 512], F32, tag="xc", name="xc", bufs=2)
                    nc.scalar.dma_start(out=rr(xc[:]),
                                        in_=rr(xT_in[:, c * 512:(c + 1) * 512]))
                    pe = mmtile()
                    mmr(out=pe[:64, :], lhsT=embw[:], rhs=xc[:],
                        start=True, stop=True)
                    f0c = wk.tile([64, 512], BF16, tag="f0c", name="f0c",
                                  bufs=2)
                    nc.scalar.activation(out=f0c[:], in_=pe[:64, :],
                                         func=AF.Relu, bias=embb[:])
                    nc.scalar.dma_start(out=f0T_d[:, c * 512:(c + 1) * 512],
                                          in_=f0c[:])
                    f0cs.append(f0c)
                pt = psb()
                for cc in range(2):
                    for t4 in range(4):
                        blk = cc * 4 + t4
                        nc.tensor.matmul(
                            out=pt[0:128, blk * 64:(blk + 1) * 64],
                            lhsT=f0cs[cc][:, t4 * 128:(t4 + 1) * 128],
                            rhs=identb[0:64, 0:64],
                            is_transpose=True, start=True, stop=True)
                nc.scalar.activation(
                    out=stage[:],
                    in_=pt[0:128, 0:512].rearrange("p (t c) -> p t c", c=64),
                    func=AF.Copy)
                nc.sync.dma_start(
                    out=tab0_d.rearrange("(t p) c -> p t c", p=P)[
                        :, g * 8:(g + 1) * 8, 0:64],
                    in_=stage[:])

        def knn_sel(lhsT_ap, rhs_ap, ncand, nwin, tag):
            """top-32 of ncand candidates for 128 queries: ACT copies each
            PSUM window to SBUF bf16, window top-8 on DVE max8/max_index in
            bf16 (2x scan rate), then 4-round refine + index extraction.
            Returns (P, 32) i16 global candidate indices."""
            ncw = ncand // nwin  # window width
            candV = wk.tile([P, 8 * nwin], F32, tag=f"cV{tag}", name=f"cV{tag}")
            candI = wk.tile([P, 8 * nwin], U16, tag=f"cI{tag}", name=f"cI{tag}")
            for c in range(nwin):
                pm = psS.tile([P, 1024], F32, tag="sel", name="sel")
                for hh in range(ncw // 512):
                    nc.tensor.matmul(
                        out=pm[:, hh * 512:(hh + 1) * 512], lhsT=lhsT_ap,
                        rhs=rhs_ap[:, c * ncw + hh * 512:
                                   c * ncw + (hh + 1) * 512],
                        start=True, stop=True)
                nc.vector.max(out=candV[:, c * 8:(c + 1) * 8],
                              in_=pm[:, 0:ncw])
                nc.vector.max_index(out=candI[:, c * 8:(c + 1) * 8],
                                    in_max=candV[:, c * 8:(c + 1) * 8],
                                    in_values=pm[:, 0:ncw])
            nc.vector.tensor_tensor(out=candI[:], in0=candI[:],
                                    in1=offs[:, 0:8 * nwin], op=ALU.add)
            return knn_refine(candV, candI, nwin, tag)

        def knn_refine(candV, candI, nwin, tag):
            """top-32 of the 8*nwin candidates -> (P,32) i16 indices.
            Packs the global index into the low 14 mantissa bits of each
            candidate value (ties within 2^-9 relative resolve by index -
            below the bf16 coordinate noise), so one 4-round max8 +
            match_replace pass yields the indices directly."""
            candIw = wk.tile([P, 8 * nwin], F32, tag=f"cIf{tag}",
                             name=f"cIf{tag}")
            nc.vector.tensor_copy(out=candIw[:].bitcast(U32), in_=candI[:])
            candP = wk.tile([P, 8 * nwin], F32, tag=f"cVw{tag}",
                            name=f"cVw{tag}")
            nc.vector.tensor_scalar(out=candP[:].bitcast(U32),
                                    in0=candV[:].bitcast(U32),
                                    scalar1=mskhi[:, 0:1], scalar2=None,
                                    op0=ALU.bitwise_and)
            nc.vector.tensor_tensor(out=candP[:].bitcast(U32),
                                    in0=candP[:].bitcast(U32),
                                    in1=candIw[:].bitcast(U32),
                                    op=ALU.bitwise_or)
            selP = wk.tile([P, K1], F32, tag=f"sV{tag}", name=f"sV{tag}")
            for rn in range(4):
                rs = slice(rn * 8, (rn + 1) * 8)
                nc.vector.max(out=selP[:, rs], in_=candP[:])
                if rn < 3:
                    nc.vector.match_replace(out=candP[:],
                                            in_to_replace=selP[:, rs],
                                            in_values=candP[:],
                                            imm_value=NEG)
            selI = wk.tile([P, K1], F32, tag=f"sI{tag}", name=f"sI{tag}")
            nc.vector.tensor_scalar(out=selI[:].bitcast(U32),
                                    in0=selP[:].bitcast(U32),
                                    scalar1=msklo[:, 0:1], scalar2=None,
                                    op0=ALU.bitwise_and)
            nbr16 = wk.tile([P, K1], I16, tag=f"nb{tag}", name=f"nb{tag}")
            nc.vector.tensor_copy(out=nbr16[:], in_=selI[:].bitcast(U32))
            return nbr16

        # ---- SA1 ----
        def sa1_sel(qt):
            nbr16 = knn_sel(q1t[:, qt * P:(qt + 1) * P], bigT[:], N, 16, "s1")
            return wrap_idx(nbr16[:], ib1_ds[qt][:, :], "idxt1", bufs=3)

        def sa1_gather(idxt):
            halves = []
            for hh in range(2):
                g = wk.tile([P, K1 // 2, TAB0_W], BF16, tag="gn", name="gn")
                for k in range(2):
                    kk = hh * 2 + k
                    nc.gpsimd.dma_gather(g[:, k * 8:(k + 1) * 8, :], tab0_d[:],
                                         idxt[:, kk * 64:(kk + 1) * 64],
                                         1024, 1024, TAB0_W)
                halves.append(g)
            return halves

        def sa1_mlp(qt, gnh):
            nq = nqb1[qt % 2]
            nc.scalar.activation(
                out=nq[0:3, :].rearrange("r (j q) -> r j q", q=P),
                in_=nq1sb[0:3, qt * P:(qt + 1) * P].unsqueeze(1).to_broadcast(
                    [3, 4, P]),
                func=AF.Copy)
            acc = wk.tile([P, P], BF16, tag="sa1acc", name="sa1acc")
            for c in range(8):
                gn = gnh[c // 4]
                cl = c % 4
                pg = psb()
                for j in range(4):
                    nc.tensor.matmul(out=pg[0:68, j * 128:(j + 1) * 128],
                                     lhsT=gn[:, cl * 4 + j, 0:68],
                                     rhs=identb[:], is_transpose=True,
                                     start=True, stop=True)
                gt = wk.tile([68, 512], BF16, tag="gt", name="gt", bufs=2)
                nc.scalar.activation(out=gt[:], in_=pg[0:68, :], func=AF.Copy)
                pz = mmtile()
                nc.tensor.matmul(out=pz[:], lhsT=w1aug[:], rhs=gt[:],
                                 start=True, stop=False)
                nc.tensor.matmul(out=pz[:], lhsT=w1q[:], rhs=nq[:],
                                 start=False, stop=True)
                h1 = wk.tile([P, 512], BF16, tag="h1", name="h1", bufs=2)
                nc.scalar.activation(out=h1[:], in_=pz[:], func=AF.Relu)
                pz2 = mmtile()
                nc.tensor.matmul(out=pz2[:], lhsT=w2sb[:], rhs=h1[:],
                                 start=True, stop=True)
                red = wk.tile([P, P], BF16, tag="sa1red", name="sa1red",
                              bufs=2)
                nc.vector.tensor_reduce(
                    out=red[:], in_=pz2[:].rearrange("f (s q) -> f q s", q=P),
                    axis=AX.X, op=ALU.max)
                if c == 0:
                    nc.vector.tensor_copy(out=acc[:], in_=red[:])
                else:
                    nc.vector.tensor_tensor(out=acc[:], in0=acc[:],
                                            in1=red[:], op=ALU.max)
            nc.scalar.activation(out=rr(feat1T[:, qt * P:(qt + 1) * P]),
                                 in_=acc[:], func=AF.Relu, bias=b2sb[:])

        def tab1_write():
            stage1 = wk.tile([P, T1, TAB1_W], BF16, tag="stage1", name="stage1",
                             bufs=1)
            for t in range(T1):
                pf = transf(feat1T[:, t * P:(t + 1) * P])
                nc.scalar.activation(out=stage1[:, t, 0:128], in_=pf,
                                     func=AF.Copy)
            nc.vector.tensor_copy(
                out=stage1[:, :, 128:131],
                in_=x1pm[:].rearrange("p (t c) -> p t c", c=3))
            nc.vector.memset(stage1[:, :, 131:TAB1_W], 0.0)
            for q4 in range(4):
                ts_ = slice(q4 * 2, (q4 + 1) * 2)
                nc.sync.dma_start(
                    out=tab1_d.rearrange("(t p) c -> p t c", p=P)[:, ts_, :],
                    in_=stage1[:, ts_, :])

        # ---- SA2 ----
        def sa2_sel(t2):
            pm2 = psS.tile([P, 1024], F32, tag="sel", name="sel")
            for hh in range(2):
                nc.tensor.matmul(out=pm2[:, hh * 512:(hh + 1) * 512],
                                 lhsT=q2t[:, t2 * P:(t2 + 1) * P],
                                 rhs=c1t[:, hh * 512:(hh + 1) * 512],
                                 start=True, stop=True)
            selV2 = wk.tile([P, K2], F32, tag="selV2", name="selV2")
            selI2 = wk.tile([P, K2], U16, tag="selI2", name="selI2")
            for rn in range(4):
                rs = slice(rn * 8, (rn + 1) * 8)
                nc.vector.max(out=selV2[:, rs], in_=pm2[:])
                nc.vector.max_index(out=selI2[:, rs], in_max=selV2[:, rs],
                                    in_values=pm2[:])
                if rn < 3:
                    nc.vector.match_replace(out=pm2[:],
                                            in_to_replace=selV2[:, rs],
                                            in_values=pm2[:], imm_value=NEG)
            nbr2 = wk.tile([P, K2], I16, tag="nbr2", name="nbr2")
            nc.vector.tensor_copy(out=nbr2[:], in_=selI2[:])
            return wrap_idx(nbr2[:], ib2_ds[t2][:, :], "idxt2")

        def sa2_gather(idxt2):
            halves = []
            for hh in range(2):
                g = wk.tile([P, K2 // 2, TAB1_W], BF16, tag="gn2",
                            name="gn2")
                for k in range(2):
                    kk = hh * 2 + k
                    nc.gpsimd.dma_gather(g[:, k * 8:(k + 1) * 8, :], tab1_d[:],
                                         idxt2[:, kk * 64:(kk + 1) * 64],
                                         1024, 1024, TAB1_W)
                halves.append(g)
            return halves

        def sa2_mlp(t2, gn2h):
            nq2 = nqb2[t2 % 2]
            nc.scalar.activation(
                out=nq2[0:3, :].rearrange("r (j q) -> r j q", q=P),
                in_=nq2sb[0:3, t2 * P:(t2 + 1) * P].unsqueeze(1).to_broadcast(
                    [3, 4, P]),
                func=AF.Copy)
            acc2 = [wk.tile([P, P], BF16, tag=f"sa2acc{h}", name=f"sa2acc{h}")
                    for h in range(2)]
            for c in range(8):
                gn2 = gn2h[c // 4]
                cl = c % 4
                pga = psb()
                pgb = psb()
                for j in range(4):
                    nc.tensor.matmul(out=pga[:, j * 128:(j + 1) * 128],
                                     lhsT=gn2[:, cl * 4 + j, 0:128],
                                     rhs=identb[:], is_transpose=True,
                                     start=True, stop=True)
                    nc.tensor.matmul(out=pgb[0:3, j * 128:(j + 1) * 128],
                                     lhsT=gn2[:, cl * 4 + j, 128:131],
                                     rhs=identb[:], is_transpose=True,
                                     start=True, stop=True)
                gta = wk.tile([P, 512], BF16, tag="gta", name="gta", bufs=2)
                gtb = wk.tile([3, 512], BF16, tag="gtb", name="gtb", bufs=2)
                nc.scalar.activation(out=gta[:], in_=pga[:], func=AF.Copy)
                nc.scalar.activation(out=gtb[:], in_=pgb[0:3, :], func=AF.Copy)
                h1c = []
                for h in range(2):
                    pz = mmtile()
                    nc.tensor.matmul(out=pz[:], lhsT=v1A[h][:], rhs=gta[:],
                                     start=True, stop=False)
                    nc.tensor.matmul(out=pz[:], lhsT=v1rel[h][:], rhs=gtb[:],
                                     start=False, stop=False)
                    nc.tensor.matmul(out=pz[:], lhsT=v1q[h][:], rhs=nq2[:],
                                     start=False, stop=True)
                    hh_ = wk.tile([P, 512], BF16, tag=f"h1c{h}", name=f"h1c{h}",
                                  bufs=2)
                    nc.scalar.activation(out=hh_[:], in_=pz[:], func=AF.Relu)
                    h1c.append(hh_)
                for h in range(2):
                    pz = mmtile()
                    nc.tensor.matmul(out=pz[:], lhsT=v2sb[0][h][:],
                                     rhs=h1c[0][:], start=True, stop=False)
                    nc.tensor.matmul(out=pz[:], lhsT=v2sb[1][h][:],
                                     rhs=h1c[1][:], start=False, stop=True)
                    red = wk.tile([P, P], BF16, tag="sa2red", name="sa2red",
                                  bufs=2)
                    nc.vector.tensor_reduce(
                        out=red[:],
                        in_=pz[:].rearrange("f (s q) -> f q s", q=P),
                        axis=AX.X, op=ALU.max)
                    if c == 0:
                        nc.vector.tensor_copy(out=acc2[h][:], in_=red[:])
                    else:
                        nc.vector.tensor_tensor(out=acc2[h][:], in0=acc2[h][:],
                                                in1=red[:], op=ALU.max)
            for h in range(2):
                nc.scalar.activation(out=feat2T[h][:, t2 * P:(t2 + 1) * P],
                                     in_=acc2[h][:], func=AF.Relu,
                                     bias=c2sb[:, h:h + 1])

        def tab2_write():
            stage2 = wk.tile([P, T2, 256], BF16, tag="stage2", name="stage2",
                             bufs=1)
            for t2 in range(T2):
                for h in range(2):
                    pf = transf(feat2T[h][:, t2 * P:(t2 + 1) * P])
                    nc.scalar.activation(
                        out=stage2[:, t2, h * 128:(h + 1) * 128],
                        in_=pf, func=AF.Copy)
            for q2 in range(2):
                nc.sync.dma_start(
                    out=tab2_d.rearrange("(t p) c -> p t c", p=P)[
                        :, q2:q2 + 1, :],
                    in_=stage2[:, q2:q2 + 1, :])

        # ---- FP2 ----
        def fp2_sel():
            v8f = wk.tile([P, T1, 8], F32, tag="v8f", name="v8f", bufs=1)
            p8f = wk.tile([P, T1, 8], U16, tag="p8f", name="p8f", bufs=1)
            for qt in range(T1):
                pm3 = psS.tile([P, 1024], F32, tag="sel", name="sel")
                nc.tensor.matmul(out=pm3[:, 0:S2],
                                 lhsT=c1t[:, qt * P:(qt + 1) * P],
                                 rhs=rF2[:], start=True, stop=True)
                nc.vector.max(out=v8f[:, qt, :], in_=pm3[:, 0:S2])
                nc.vector.max_index(out=p8f[:, qt, :], in_max=v8f[:, qt, :],
                                    in_values=pm3[:, 0:S2])
            wn2 = wk.tile([P, T1, 3], F32, tag="wn2", name="wn2", bufs=1)
            interp_weights(sq1pm[:], v8f[:, :, 0:3], T1, wn2[:])
            p3f = wk.tile([P, T1 * 3], I16, tag="p3f", name="p3f", bufs=1)
            nc.vector.tensor_copy(
                out=p3f[:].rearrange("p (t j) -> p t j", j=3),
                in_=p8f[:, :, 0:3])
            idxtf2 = wrap_idx(p3f[:], ibf2_d[:, :], "idxtf2")
            return idxtf2, wn2

        def fp2_gather(idxtf2):
            halves = []
            for hh in range(2):
                gi2 = wk.tile([P, T1 // 2, 3, 256], BF16, tag="gi2",
                              name="gi2")
                gi2v = gi2[:].rearrange("p t j c -> p (t j) c")
                # 24 (t,j) row-slots split as 12+12 across the two halves
                for k in range(3):
                    nc.gpsimd.dma_gather(
                        gi2v[:, k * 4:(k + 1) * 4, :], tab2_d[:],
                        idxtf2[:, hh * 96 + k * 32:hh * 96 + (k + 1) * 32],
                        512, 512, 256)
                halves.append(gi2)
            return halves

        def fp2_mlp(gi2h, wn2):
            it2pm = wk.tile([P, T1, 256], BF16, tag="it2pm", name="it2pm",
                            bufs=1)
            tmp2 = wk.tile([P, T1 // 2, 256], BF16, tag="it2tmp",
                           name="it2tmp", bufs=1)
            for hh in range(2):
                gi2 = gi2h[hh]
                ts_ = slice(hh * (T1 // 2), (hh + 1) * (T1 // 2))
                def wb2(j):
                    return wn2[:, ts_, j:j + 1].to_broadcast(
                        [P, T1 // 2, 256])
                nc.vector.tensor_tensor(out=it2pm[:, ts_, :],
                                        in0=gi2[:, :, 1, :], in1=wb2(1),
                                        op=ALU.mult)
                nc.vector.tensor_tensor(out=tmp2[:], in0=gi2[:, :, 2, :],
                                        in1=wb2(2), op=ALU.mult)
                nc.vector.tensor_tensor(out=it2pm[:, ts_, :],
                                        in0=it2pm[:, ts_, :], in1=tmp2[:],
                                        op=ALU.add)
                nc.vector.tensor_tensor(out=tmp2[:], in0=gi2[:, :, 0, :],
                                        in1=wb2(0), op=ALU.mult)
                nc.vector.tensor_tensor(out=it2pm[:, ts_, :],
                                        in0=it2pm[:, ts_, :], in1=tmp2[:],
                                        op=ALU.add)
            itT2 = [wk.tile([P, S1], BF16, tag=f"itT2{h}", name=f"itT2{h}",
                            bufs=1) for h in range(2)]
            for h in range(2):
                for h4 in range(2):
                    pf = psb()
                    for tl in range(4):
                        t = h4 * 4 + tl
                        nc.tensor.matmul(
                            out=pf[0:128, tl * 128:(tl + 1) * 128],
                            lhsT=it2pm[:, t, h * 128:(h + 1) * 128],
                            rhs=identb[:], is_transpose=True,
                            start=True, stop=True)
                    nc.scalar.activation(out=itT2[h][:, h4 * 512:(h4 + 1) * 512],
                                         in_=pf[0:128, 0:512], func=AF.Copy)
            for c in range(2):
                cs = slice(c * 512, (c + 1) * 512)
                pz = mmtile()
                mmr(out=pz[:], lhsT=f2w1[0][:], rhs=feat1T[:, cs],
                    start=True, stop=False)
                nc.tensor.matmul(out=pz[:], lhsT=f2w1[1][:],
                                 rhs=itT2[0][:, cs], start=False, stop=False)
                nc.tensor.matmul(out=pz[:], lhsT=f2w1[2][:],
                                 rhs=itT2[1][:, cs], start=False, stop=True)
                hf = wk.tile([P, 512], F32, tag="fp2h", name="fp2h", bufs=1)
                nc.scalar.activation(out=rr(hf[:]), in_=pz[:], func=AF.Relu,
                                     bias=f2b1[:])
                pz2 = mmtile()
                mmr(out=pz2[:], lhsT=f2w2[:], rhs=hf[:], start=True, stop=True)
                nc.scalar.activation(out=f1upT[:, cs], in_=pz2[:], func=AF.Relu,
                                     bias=f2b2[:])

        def tabf_write():
            stagef = wk.tile([P, T1, 128], BF16, tag="stagef", name="stagef",
                             bufs=1)
            for t in range(T1):
                pf = transf(f1upT[:, t * P:(t + 1) * P])
                nc.scalar.activation(out=stagef[:, t, :], in_=pf, func=AF.Copy)
            for q4 in range(4):
                ts_ = slice(q4 * 2, (q4 + 1) * 2)
                nc.sync.dma_start(
                    out=tabf_d.rearrange("(t p) c -> p t c", p=P)[:, ts_, :],
                    in_=stagef[:, ts_, :])

        # ---- FP1 ----
        def fp1_sel(g):
            v8g = wk.tile([P, GT, 8], F32, tag="v8g", name="v8g", bufs=3)
            p8g = wk.tile([P, GT, 8], U16, tag="p8g", name="p8g", bufs=3)
            for t in range(GT):
                qt = g * GT + t
                pm4 = psS.tile([P, 1024], F32, tag="sel", name="sel")
                for hh in range(2):
                    nc.tensor.matmul(out=pm4[:, hh * 512:(hh + 1) * 512],
                                     lhsT=bigT[:, qt * P:(qt + 1) * P],
                                     rhs=r2x[:, hh * 512:(hh + 1) * 512],
                                     start=True, stop=True)
                nc.vector.max(out=v8g[:, t, :], in_=pm4[:])
                nc.vector.max_index(out=p8g[:, t, :], in_max=v8g[:, t, :],
                                    in_values=pm4[:])
            wng = wk.tile([P, GT, 3], F32, tag="wng", name="wng", bufs=9)
            interp_weights(sqpm[:, g * GT:(g + 1) * GT], v8g[:, :, 0:3], GT,
                           wng[:])
            p3g = wk.tile([P, GT * 3], I16, tag="p3g", name="p3g", bufs=4)
            nc.vector.tensor_copy(
                out=p3g[:].rearrange("p (t j) -> p t j", j=3),
                in_=p8g[:, :, 0:3])
            idxtg = wrap_idx(p3g[:], ibf1_d[:, g * 192:(g + 1) * 192],
                             "idxtf1", bufs=7)
            return idxtg, wng

        def fp1_gather(g, idxtg):
            gi1 = wk.tile([P, GT, 3, 128], BF16, tag="gi1", name="gi1",
                          bufs=3)
            gi1v = gi1[:].rearrange("p t j c -> p (t j) c")
            for k in range(3):
                nc.gpsimd.dma_gather(gi1v[:, k * 8:(k + 1) * 8, :], tabf_d[:],
                                     idxtg[:, k * 64:(k + 1) * 64],
                                     1024, 1024, 128)
            f0Tc = wk.tile([64, GT * 128], BF16, tag="f0Tc", name="f0Tc",
                           bufs=2)
            nc.sync.dma_start(out=f0Tc[:],
                              in_=f0T_d[:, g * GT * P:(g + 1) * GT * P])
            return gi1, f0Tc

        def fp1_mlp(g, gif, wng):
            gi1, f0Tc = gif
            it1pm = wk.tile([P, GT, 128], BF16, tag="it1pm", name="it1pm",
                            bufs=1)
            tmp1 = wk.tile([P, GT, 128], BF16, tag="ittmp", name="ittmp",
                           bufs=1)
            def wb(j):
                return wng[:, :, j:j + 1].to_broadcast([P, GT, 128])
            nc.vector.tensor_tensor(out=it1pm[:], in0=gi1[:, :, 1, :],
                                    in1=wb(1), op=ALU.mult)
            nc.vector.tensor_tensor(out=tmp1[:], in0=gi1[:, :, 2, :],
                                    in1=wb(2), op=ALU.mult)
            nc.vector.tensor_tensor(out=it1pm[:], in0=it1pm[:], in1=tmp1[:],
                                    op=ALU.add)
            nc.vector.tensor_tensor(out=tmp1[:], in0=gi1[:, :, 0, :],
                                    in1=wb(0), op=ALU.mult)
            nc.vector.tensor_tensor(out=it1pm[:], in0=it1pm[:], in1=tmp1[:],
                                    op=ALU.add)
            itT1 = wk.tile([P, GT * 128], BF16, tag="itT1", name="itT1",
                           bufs=1)
            for h4 in range(2):
                pf = psb()
                for tl in range(4):
                    t = h4 * 4 + tl
                    nc.tensor.matmul(out=pf[0:128, tl * 128:(tl + 1) * 128],
                                     lhsT=it1pm[:, t, :],
                                     rhs=identb[:], is_transpose=True,
                                     start=True, stop=True)
                nc.scalar.activation(out=itT1[:, h4 * 512:(h4 + 1) * 512],
                                     in_=pf[0:128, 0:512], func=AF.Copy)
            fus = []
            for c in range(2):
                cs = slice(c * 512, (c + 1) * 512)
                gcs = slice(g * GT * P + c * 512, g * GT * P + (c + 1) * 512)
                pz = mmtile()
                nc.tensor.matmul(out=pz[:], lhsT=f1w1b[:], rhs=itT1[:, cs],
                                 start=True, stop=False)
                nc.tensor.matmul(out=pz[:], lhsT=f1w1a[:], rhs=f0Tc[:, cs],
                                 start=False, stop=True)
                hf = wk.tile([P, 512], F32, tag="fp1h", name="fp1h", bufs=2)
                nc.scalar.activation(out=rr(hf[:]), in_=pz[:], func=AF.Relu,
                                     bias=f1b1[:])
                pz2 = mmtile()
                mmr(out=pz2[:], lhsT=f1w2[:], rhs=hf[:], start=True, stop=True)
                fu = wk.tile([P, 512], BF16, tag="fuc", name="fuc", bufs=3)
                nc.scalar.activation(out=fu[:], in_=pz2[:], func=AF.Relu,
                                     bias=f1b2[:])
                nc.sync.dma_start(out=fuT_d[:, gcs], in_=fu[:])
                fus.append(fu)
            return fus

        def fp1_tail(g, fus):
            """global-max accumulation (DVE) - emitted after sel(g) so it
            never head-of-line blocks the selection scans."""
            for c, fu in enumerate(fus):
                if g == 0 and c == 0:
                    nc.vector.tensor_reduce(out=gfacc[:], in_=fu[:],
                                            axis=AX.X, op=ALU.max)
                else:
                    red1 = wk.tile([P, 1], F32, tag="gfred1", name="gfred1",
                                   bufs=2)
                    nc.vector.tensor_reduce(out=red1[:], in_=fu[:],
                                            axis=AX.X, op=ALU.max)
                    nc.vector.tensor_tensor(out=gfacc[:], in0=gfacc[:],
                                            in1=red1[:], op=ALU.max)

        # ---- head ----
        def head():
            pc = mmtile()
            nc.tensor.matmul(out=pc[:, 0:1], lhsT=hw1b[:], rhs=gfacc[:],
                             start=True, stop=True)
            nc.vector.tensor_tensor(out=biasH[:], in0=pc[:, 0:1], in1=hb1[:],
                                    op=ALU.add)
            for g in range(8):
                for c4 in range(4):
                    c = g * 4 + c4
                    cs = slice(c * 512, (c + 1) * 512)
                    fuc = wk.tile([P, 512], BF16, tag="hfuc", name="hfuc",
                                  bufs=2)
                    nc.sync.dma_start(out=fuc[:], in_=fuT_d[:, cs])
                    pzs = psS.tile([P, 1024], F32, tag="sel", name="sel")
                    pz = pzs[:, 0:512]
                    nc.tensor.matmul(out=pz, lhsT=hw1a[:], rhs=fuc[:],
                                     start=True, stop=True)
                    h1 = wk.tile([P, 512], F32, tag="hh1", name="hh1", bufs=2)
                    if c % 2 == 0:
                        nc.scalar.activation(out=rr(h1[:]), in_=pz,
                                             func=AF.Relu, bias=biasH[:])
                    else:
                        nc.vector.scalar_tensor_tensor(
                            out=rr(h1[:]), in0=pz, scalar=biasH[:],
                            in1=zroP[:], op0=ALU.add, op1=ALU.max)
                    pz2 = mmtile()
                    mmr(out=pz2[:64, :], lhsT=hw2[:], rhs=h1[:],
                        start=True, stop=True)
                    h2 = wk.tile([64, 512], F32, tag="hh2", name="hh2", bufs=2)
                    nc.vector.scalar_tensor_tensor(
                        out=rr(h2[:]), in0=pz2[:64, :], scalar=hb2[:],
                        in1=zroP[0:64, :], op0=ALU.add, op1=ALU.max)
                    pz3t = psT.tile([P, 512], F32, tag="trans", name="trans")
                    pz3 = pz3t[:]
                    mmr(out=pz3[:13, :], lhsT=hw3[:], rhs=h2[:],
                        start=True, stop=True)
                    oT = wk.tile([13, 512], F32, tag="hoT", name="hoT", bufs=2)
                    nc.scalar.activation(out=oT[:], in_=pz3[:13, :],
                                         func=AF.Identity, bias=hb3[:])
                    nc.sync.dma_start(
                        out=out_d[:, c * 512:(c + 1) * 512], in_=oT[:])

        # ---------------- global software pipeline ----------------
        # DVE runs the selection stream back-to-back; PE/ACT/Pool/DMA run the
        # embed->SA1->SA2->FP2->FP1->head MLP chain in the gaps.
        idx1 = {}
        idx1[0] = sa1_sel(0)
        stage01()
        idx1[1] = sa1_sel(1)
        idx1[2] = sa1_sel(2)
        (c1t, r2x, q2t, rF2, nq1sb, nq2sb, sqpm, sq1pm, x1pm) = late_loads()
        (b2sb, c2sb, f2b1, f2w2, f2b2, f1b1, f1w2, f1b2, hb1, hw2, hb2, hw3,
         hb3, f2w1, f1w1a, f1w1b, hw1a, hw1b, w1aug, w1q, v1A, v1rel, v1q,
         v2sb, w2sb) = late_weights()
        gn1 = {}
        gn1[0] = sa1_gather(idx1[0])
        gn1[1] = sa1_gather(idx1[1])
        for qt in range(3, T1):
            idx1[qt] = sa1_sel(qt)
            gn1[qt - 1] = sa1_gather(idx1[qt - 1])
            sa1_mlp(qt - 3, gn1[qt - 3])
        idx2 = {}
        idx2[0] = sa2_sel(0)
        gn1[T1 - 1] = sa1_gather(idx1[T1 - 1])
        sa1_mlp(T1 - 3, gn1[T1 - 3])
        idx2[1] = sa2_sel(1)
        sa1_mlp(T1 - 2, gn1[T1 - 2])
        idxf2, wn2 = fp2_sel()
        sa1_mlp(T1 - 1, gn1[T1 - 1])
        tab1_write()
        idxg = {}
        wng = {}
        g2 = {}
        g2[0] = sa2_gather(idx2[0])
        idxg[0], wng[0] = fp1_sel(0)
        g2[1] = sa2_gather(idx2[1])
        sa2_mlp(0, g2[0])
        idxg[1], wng[1] = fp1_sel(1)
        sa2_mlp(1, g2[1])
        tab2_write()
        gi2h = fp2_gather(idxf2)
        idxg[2], wng[2] = fp1_sel(2)
        fp2_mlp(gi2h, wn2)
        tabf_write()
        gi = {}
        gi[0] = fp1_gather(0, idxg[0])
        gi[1] = fp1_gather(1, idxg[1])
        for g in range(3, NG):
            idxg[g], wng[g] = fp1_sel(g)
            gi[g - 1] = fp1_gather(g - 1, idxg[g - 1])
            fp1_tail(g - 3, fp1_mlp(g - 3, gi[g - 3], wng[g - 3]))
        gi[NG - 1] = fp1_gather(NG - 1, idxg[NG - 1])
        fp1_tail(NG - 3, fp1_mlp(NG - 3, gi[NG - 3], wng[NG - 3]))
        fp1_tail(NG - 2, fp1_mlp(NG - 2, gi[NG - 2], wng[NG - 2]))
        fp1_tail(NG - 1, fp1_mlp(NG - 1, gi[NG - 1], wng[NG - 1]))
        head()

    return nc


# ---------------------------------------------------------------- host side
_CACHED_NC = None


def _get_nc():
    global _CACHED_NC
    if _CACHED_NC is None:
        nc = build_nc()
        nc.finalize()
        _CACHED_NC = nc
    return _CACHED_NC


def _per_core_inputs(b, inputs):
    import ml_dtypes
    bf16 = ml_dtypes.bfloat16
    x = np.ascontiguousarray(np.asarray(inputs["x"][b]), dtype=np.float32)
    i1 = np.asarray(inputs["idx_s1"][b]).astype(np.int64)
    i2 = np.asarray(inputs["idx_s2"][b]).astype(np.int64)
    xyz = x[:, 0:3].astype(np.float32)
    sq = (xyz * xyz).sum(1)                      # |x|^2  (N,)
    x1 = xyz[i1]                                 # (S1,3)
    sq1 = (x1 * x1).sum(1)
    x2 = x1[i2]                                  # (S2,3)
    sq2 = (x2 * x2).sum(1)
    one = np.ones
    zer = np.zeros
    f32 = lambda a: np.ascontiguousarray(np.asarray(a), dtype=np.float32)
    b16 = lambda a: np.ascontiguousarray(np.asarray(a, dtype=np.float32)
                                         .astype(bf16))
    # Scores are shifted by -|q|^2 (the spare row in each table) so the PE
    # emits ~ -d^2 + delta_q: small values that survive the bf16 PSUM->SBUF
    # copy with ~2^-9 relative error.  delta_q = |q|^2 - f32(bf16(|q|^2)) is
    # the query-constant residue; sqpm/sq1pm carry it for the interp weights
    # (d^2 = delta_q - score').
    bigT = np.concatenate([xyz.T, -sq[None], one((1, N))], 0)
    q1t = np.concatenate([2 * x1.T, one((1, S1)), -sq1[None]], 0)
    c1t = np.concatenate([x1.T, -sq1[None], one((1, S1))], 0)
    r2x = np.concatenate([2 * x1.T, one((1, S1)), -sq1[None]], 0)
    q2t = np.concatenate([2 * x2.T, one((1, S2)), -sq2[None]], 0)
    rF2 = np.concatenate([2 * x2.T, one((1, S2)), -sq2[None]], 0)
    dlt = sq - np.asarray(sq.astype(bf16), dtype=np.float32)
    dlt1 = sq1 - np.asarray(sq1.astype(bf16), dtype=np.float32)
    xyzb = np.concatenate([xyz.reshape(NT, P, 3), np.zeros((NT, P, 1))],
                          2).transpose(1, 0, 2).reshape(P, NT * 4)
    w1 = f32(inputs["sa1_w1"])                   # (67,128): [rel(3);feat(64)]
    b1 = f32(inputs["sa1_b1"]).reshape(1, 128)
    w1aug = np.concatenate([w1[3:67], w1[0:3], zer((1, 128))], 0)
    w1q = np.concatenate([w1[0:3], b1], 0)
    v1 = f32(inputs["sa2_w1"])                   # (131,256)
    c1r = f32(inputs["sa2_b1"]).reshape(1, 256)
    v1q = np.concatenate([v1[0:3], c1r], 0)
    return {
        "xT": np.ascontiguousarray(x.T),
        "bigT": b16(bigT),
        "q1t": b16(q1t),
        "c1t": b16(c1t),
        "r2x": b16(r2x),
        "q2t": b16(q2t),
        "rF2": b16(rF2),
        "nq1": b16(-x1.T),
        "nq2": b16(-x2.T),
        "sqpm": f32(dlt.reshape(NT, P).T),
        "sq1pm": f32(dlt1.reshape(T1, P).T),
        "xyzb": b16(xyzb),
        "x1pm": b16(x1.reshape(T1, P, 3).transpose(1, 0, 2).reshape(P, T1 * 3)),
        "w1augb": b16(w1aug),
        "w1qb": b16(w1q),
        "v1Ab": b16(v1[3:131]),
        "v1relb": b16(v1[0:3]),
        "v1qb": b16(v1q),
        "v2b": b16(inputs["sa2_w2"]),
        "w2b": b16(inputs["sa1_w2"]),
        "f2w1b": b16(f32(inputs["fp2_w1"])[128:384]),
        "f1w1ab": b16(f32(inputs["fp1_w1"])[0:64]),
        "f1w1bb": b16(f32(inputs["fp1_w1"])[64:192]),
        "hw1ab": b16(f32(inputs["head_w1"])[0:128]),
        "embw": f32(inputs["embed_w"]),
        "embb": f32(inputs["embed_b"]).reshape(64, 1),
        "b2": f32(inputs["sa1_b2"]).reshape(128, 1),
        "c2": np.ascontiguousarray(f32(inputs["sa2_b2"]).reshape(2, 128).T),
        "f2w1": f32(inputs["fp2_w1"]),
        "f2b1": f32(inputs["fp2_b1"]).reshape(128, 1),
        "f2w2": f32(inputs["fp2_w2"]),
        "f2b2": f32(inputs["fp2_b2"]).reshape(128, 1),
        "f1w1": f32(inputs["fp1_w1"]),
        "f1b1": f32(inputs["fp1_b1"]).reshape(128, 1),
        "f1w2": f32(inputs["fp1_w2"]),
        "f1b2": f32(inputs["fp1_b2"]).reshape(128, 1),
        "hw1": f32(inputs["head_w1"]),
        "hb1": f32(inputs["head_b1"]).reshape(128, 1),
        "hw2": f32(inputs["head_w2"]),
        "hb2": f32(inputs["head_b2"]).reshape(64, 1),
        "hw3": f32(inputs["head_w3"]),
        "hb3": f32(inputs["head_b3"]).reshape(13, 1),
    }


def run(inputs, trace=False, **kw):
    nc = _get_nc()
    B = inputs["x"].shape[0]
    in_maps = [_per_core_inputs(b, inputs) for b in range(B)]
    res = run_bass_kernel_spmd(nc, in_maps, core_ids=list(range(B)),
                               trace=trace, **kw)
    out = np.stack([np.ascontiguousarray(res.results[b]["out"].T)
                    for b in range(B)])
    return out, res


def kernel(**inputs):
    return run(inputs)[0]


if __name__ == "__main__":
    build_nc()
    print("built ok")



# revision 44
# speedup vs baseline: 1.0455x; 1.0455x over previous
"""PointNet++-lite segmentation on 8 Trainium2 cores (batch-parallel, one
point cloud per core). Self-contained: hardcodes shapes from the problem spec.

Per-core pipeline (all on device):
  embed MLP -> SA1 (KNN top-32 of 16384, gather, 2-layer MLP, max-pool)
  -> SA2 (KNN top-32 of 1024) -> FP2/FP1 (3-NN inverse-distance interp)
  -> global-max head MLP -> (16384, 13) logits.

Performance design (HW-measured; DVE scans + SWDGE gather descriptor
generation are the two ~equal critical resources, ~60% busy each):
- KNN ranking m' = 2 q.x - |x|^2 - bf16(|q|^2) on the PE from bf16
  host-precomputed coordinate tables (1 cyc/row).  The -|q|^2 shift rides
  the spare table row for free and centers scores at ~-d^2 + delta_q;
  sqpm/sq1pm carry delta_q = |q|^2 - f32(bf16(|q|^2)) so the FP interp
  weights recover d^2 = delta_q - score exactly to f32.
- Top-32 selection on the DVE: max8/max_index over 1024-wide PSUM windows
  (measured 1.19 ns/el regardless of dtype - bf16 SBUF copies do NOT get a
  2x mode for InstMax, don't re-add them), then the global index is packed
  into the low 14 mantissa bits of each candidate so ONE 4-round
  max8/match_replace pass extracts values+indices together.
- The selection stream is software-pipelined: sel(i+k) emitted ahead of
  mlp(i); FP1 runs sel(g) / gather(g-1) / mlp(g-3) per iteration.  KEEP
  this emission order - in-order engine queues head-of-line block, so any
  scheme that makes mlp ACT/PE ops depend on the current group's DVE ops
  (e.g. PE-side diag(w) interp matmuls fed by ACT) measures 20-30% SLOWER
  on HW despite lower DVE work.
- FP interp weighted-sums are batched tensor_tensor ops over all 8 tiles
  per neighbor j (5 wide ops/group instead of 24 narrow ones).
- SA1/SA2 MLPs, gather tables (tab0/1/2/f, f0T) and neighbor tiles in bf16
  (1 cyc/row matmuls + transposes, half DMA); FP/head MLPs in fp32r (TF32;
  producers write rounded F32R APs per walrus' requirement).
- Logits are written feature-major (13, N) and transposed on host - kills
  4 PE transposes + a DVE copy + staging per head chunk.
- Bulk weight/table loads are emitted AFTER the first sel/stage01 so the
  Sync queue serves the latency-critical wrap/tab0 traffic first; stage01
  input loads + f0T writes ride the ACT queue.
- Run-to-run HW variance is ~10-20% (power throttling ~60% of the time at
  50% util limit); judge changes by min-of-3, not single runs.
"""

from contextlib import ExitStack

import numpy as np

import concourse.bass as bass
import concourse.mybir as mybir
from concourse.bacc import Bacc
from concourse.bass_utils import run_bass_kernel_spmd
from concourse.masks import make_identity
from concourse.tile import TileContext

F32 = mybir.dt.float32
F32R = mybir.dt.float32r
BF16 = mybir.dt.bfloat16
U16 = mybir.dt.uint16
U8 = mybir.dt.uint8
I16 = mybir.dt.int16
U32 = mybir.dt.uint32
AF = mybir.ActivationFunctionType
ALU = mybir.AluOpType
AX = mybir.AxisListType

P = 128
N = 16384
S1, K1 = 1024, 32
S2, K2 = 256, 32
NCLS = 13
NEG = -3.0e38

NT = N // P        # 128 point tiles
T1 = S1 // P       # 8 SA1 query tiles
T2 = S2 // P       # 2 SA2 query tiles
GT = 8             # FP1 group size (query tiles per group)
NG = NT // GT      # 16 FP1 groups
TAB0_W = 128       # bf16 rows: [feat0(64), xyz(3), zero, pad...] 256B
TAB1_W = 256       # bf16 rows: [feat1(128), xyz1(3), pad...] 512B


def build_nc():
    nc = Bacc()

    xT_in = nc.dram_tensor("xT", [6, N], F32, kind="ExternalInput")
    # host-precomputed coordinate tables (bf16; see _per_core_inputs)
    bigT_in = nc.dram_tensor("bigT", [5, N], BF16, kind="ExternalInput")
    q1t_in = nc.dram_tensor("q1t", [5, S1], BF16, kind="ExternalInput")
    c1t_in = nc.dram_tensor("c1t", [5, S1], BF16, kind="ExternalInput")
    r2x_in = nc.dram_tensor("r2x", [5, S1], BF16, kind="ExternalInput")
    q2t_in = nc.dram_tensor("q2t", [5, S2], BF16, kind="ExternalInput")
    rF2_in = nc.dram_tensor("rF2", [5, S2], BF16, kind="ExternalInput")
    nq1_in = nc.dram_tensor("nq1", [3, S1], BF16, kind="ExternalInput")
    nq2_in = nc.dram_tensor("nq2", [3, S2], BF16, kind="ExternalInput")
    sqpm_in = nc.dram_tensor("sqpm", [P, NT], F32, kind="ExternalInput")
    sq1pm_in = nc.dram_tensor("sq1pm", [P, T1], F32, kind="ExternalInput")
    xyzb_in = nc.dram_tensor("xyzb", [P, NT * 4], BF16, kind="ExternalInput")
    x1pm_in = nc.dram_tensor("x1pm", [P, T1 * 3], BF16, kind="ExternalInput")
    # host-assembled bf16 weights for the SA1/SA2 MLPs
    w1aug_in = nc.dram_tensor("w1augb", [68, 128], BF16, kind="ExternalInput")
    w1q_in = nc.dram_tensor("w1qb", [4, 128], BF16, kind="ExternalInput")
    v1A_in = nc.dram_tensor("v1Ab", [128, 256], BF16, kind="ExternalInput")
    v1rel_in = nc.dram_tensor("v1relb", [3, 256], BF16, kind="ExternalInput")
    v1q_in = nc.dram_tensor("v1qb", [4, 256], BF16, kind="ExternalInput")
    v2_in = nc.dram_tensor("v2b", [256, 256], BF16, kind="ExternalInput")
    w2_in = nc.dram_tensor("w2b", [128, 128], BF16, kind="ExternalInput")
    f2w1b_in = nc.dram_tensor("f2w1b", [256, 128], BF16, kind="ExternalInput")
    f1w1a_in = nc.dram_tensor("f1w1ab", [64, 128], BF16, kind="ExternalInput")
    f1w1b_in = nc.dram_tensor("f1w1bb", [128, 128], BF16, kind="ExternalInput")
    hw1a_in = nc.dram_tensor("hw1ab", [128, 128], BF16, kind="ExternalInput")
    wdecl = [
        ("embw", [6, 64]), ("embb", [64, 1]),
        ("b2", [128, 1]), ("c2", [128, 2]),
        ("f2w1", [384, 128]), ("f2b1", [128, 1]), ("f2w2", [128, 128]), ("f2b2", [128, 1]),
        ("f1w1", [192, 128]), ("f1b1", [128, 1]), ("f1w2", [128, 128]), ("f1b2", [128, 1]),
        ("hw1", [256, 128]), ("hb1", [128, 1]), ("hw2", [128, 64]), ("hb2", [64, 1]),
        ("hw3", [64, 13]), ("hb3", [13, 1]),
    ]
    din = {nm: nc.dram_tensor(nm, sh, F32, kind="ExternalInput") for nm, sh in wdecl}
    out_d = nc.dram_tensor("out", [NCLS, N], F32, kind="ExternalOutput")

    tab0_d = nc.dram_tensor("tab0", [N, TAB0_W], BF16)
    tab1_d = nc.dram_tensor("tab1", [S1, TAB1_W], BF16)
    tab2_d = nc.dram_tensor("tab2", [S2, 256], BF16)
    tabf_d = nc.dram_tensor("tabf", [S1, 128], BF16)
    f0T_d = nc.dram_tensor("f0T", [64, N], BF16)
    fuT_d = nc.dram_tensor("fuT", [128, N], BF16)
    ib1_ds = [nc.dram_tensor(f"ib1_{t}", [16, 256], I16) for t in range(T1)]
    ib2_ds = [nc.dram_tensor(f"ib2_{t}", [16, 256], I16) for t in range(T2)]
    ibf2_d = nc.dram_tensor("ibf2", [16, 192], I16)
    ibf1_d = nc.dram_tensor("ibf1", [16, NT * 3 * 8], I16)

    with TileContext(nc) as tc, ExitStack() as ctx:
        cst = ctx.enter_context(tc.tile_pool(name="cst", bufs=1))
        psA = ctx.enter_context(tc.tile_pool(name="psA", bufs=2, space="PSUM"))
        psT = ctx.enter_context(tc.tile_pool(name="psT", bufs=2, space="PSUM"))
        psS = ctx.enter_context(tc.tile_pool(name="psS", bufs=2, space="PSUM"))
        wk = ctx.enter_context(tc.tile_pool(name="wk", bufs=2))

        ident = cst.tile([P, P], F32, tag="ident", name="ident")
        make_identity(nc, ident[:])
        identb = cst.tile([P, P], BF16, tag="identb", name="identb")
        make_identity(nc, identb[:])

        def mmtile():
            return psA.tile([P, 512], F32, tag="mm", name="mm")

        def rr(ap):
            return ap.bitcast(F32R)

        def mmr(out, lhsT, rhs, **kw):
            """fp32r (TF32) matmul: 1 cyc/row vs fp32's 4 for wide outputs.
            Every producer of an fp32r input must write through an F32R-typed
            out AP (walrus requires inputs 'rounded to FP32r')."""
            nc.tensor.matmul(out=out, lhsT=lhsT.bitcast(F32R),
                             rhs=rhs.bitcast(F32R), **kw)

        def transf(in_ap):
            """fp32 PE transpose: in_(p,f) -> psum slice (f,p)."""
            pt = psT.tile([P, 512], F32, tag="trans", name="trans")
            k = in_ap.shape[0]
            f = in_ap.shape[-1]
            nc.tensor.matmul(out=pt[:f, :k], lhsT=in_ap, rhs=ident[:k, :k],
                             is_transpose=True, start=True, stop=True)
            return pt[:f, :k]

        def psb():
            """bf16 view of an F32 psT bank (shares the same 2 banks)."""
            ptf = psT.tile([P, 512], F32, tag="trans", name="trans")
            return ptf[:].bitcast(BF16)[:, 0:512]

        # ---------------- constants / weights ----------------
        def load(name, src, shape, dtype=F32, rnd=False, eng=None):
            t = cst.tile(list(shape), dtype, tag=name, name=name)
            e = eng or nc.sync
            if rnd:
                e.dma_start(out=rr(t[:]), in_=rr(src))
            else:
                e.dma_start(out=t[:], in_=src)
            return t

        # stage01/sel-critical loads first; the bulk (weights, later-stage
        # tables) is emitted after sel(2) so the Sync queue serves the
        # latency-critical wrap/tab0 traffic early.
        bigT = load("bigTsb", bigT_in[:], (5, N), BF16, eng=nc.sync)
        q1t = load("q1tsb", q1t_in[:], (5, S1), BF16, eng=nc.sync)
        xyzb = load("xyzbsb", xyzb_in[:], (P, NT * 4), BF16, eng=nc.sync)
        embw = load("embw", din["embw"][:], (6, 64), rnd=True)
        embb = load("embb", din["embb"][:], (64, 1))

        def late_loads():
            return (
                load("c1tsb", c1t_in[:], (5, S1), BF16, eng=nc.sync),
                load("r2xsb", r2x_in[:], (5, S1), BF16, eng=nc.sync),
                load("q2tsb", q2t_in[:], (5, S2), BF16, eng=nc.sync),
                load("rF2sb", rF2_in[:], (5, S2), BF16, eng=nc.sync),
                load("nq1sb", nq1_in[:], (3, S1), BF16, eng=nc.sync),
                load("nq2sb", nq2_in[:], (3, S2), BF16, eng=nc.sync),
                load("sqpmsb", sqpm_in[:], (P, NT), eng=nc.sync),
                load("sq1pmsb", sq1pm_in[:], (P, T1), eng=nc.sync),
                load("x1pmsb", x1pm_in[:], (P, T1 * 3), BF16, eng=nc.sync),
            )
        def late_weights():
            b2sb = load("b2sb", din["b2"][:], (128, 1))
            c2sb = load("c2sb", din["c2"][:], (128, 2))
            f2b1 = load("f2b1", din["f2b1"][:], (128, 1))
            f2w2 = load("f2w2", din["f2w2"][:], (128, 128), rnd=True)
            f2b2 = load("f2b2", din["f2b2"][:], (128, 1))
            f1b1 = load("f1b1", din["f1b1"][:], (128, 1))
            f1w2 = load("f1w2", din["f1w2"][:], (128, 128), rnd=True)
            f1b2 = load("f1b2", din["f1b2"][:], (128, 1))
            hb1 = load("hb1", din["hb1"][:], (128, 1))
            hw2 = load("hw2", din["hw2"][:], (128, 64), rnd=True)
            hb2 = load("hb2", din["hb2"][:], (64, 1))
            hw3 = load("hw3", din["hw3"][:], (64, 13), rnd=True)
            hb3 = load("hb3", din["hb3"][:], (13, 1))
            f2w1 = [load("f2w10", din["f2w1"][0:128, :], (128, 128), rnd=True)]
            f2w1 += [load(f"f2w1{k}", f2w1b_in[(k - 1) * 128:k * 128, :],
                          (128, 128), BF16) for k in (1, 2)]
            f1w1a = load("f1w1a", f1w1a_in[:], (64, 128), BF16)
            f1w1b = load("f1w1b", f1w1b_in[:], (128, 128), BF16)
            hw1a = load("hw1a", hw1a_in[:], (128, 128), BF16)
            hw1b = load("hw1b", din["hw1"][128:256, :], (128, 128))
            w1aug = load("w1augsb", w1aug_in[:], (68, 128), BF16)
            w1q = load("w1qsb", w1q_in[:], (4, 128), BF16)
            v1A = [load(f"v1A{h}", v1A_in[:, h * 128:(h + 1) * 128],
                        (128, 128), BF16) for h in range(2)]
            v1rel = [load(f"v1rel{h}", v1rel_in[:, h * 128:(h + 1) * 128],
                          (3, 128), BF16) for h in range(2)]
            v1q = [load(f"v1q{h}", v1q_in[:, h * 128:(h + 1) * 128], (4, 128),
                        BF16) for h in range(2)]
            v2sb = [[load(f"v2{k}{h}",
                          v2_in[k * 128:(k + 1) * 128, h * 128:(h + 1) * 128],
                          (128, 128), BF16) for h in range(2)]
                    for k in range(2)]
            w2sb = load("w2sb", w2_in[:], (128, 128), BF16)
            return (b2sb, c2sb, f2b1, f2w2, f2b2, f1b1, f1w2, f1b2, hb1, hw2,
                    hb2, hw3, hb3, f2w1, f1w1a, f1w1b, hw1a, hw1b, w1aug, w1q,
                    v1A, v1rel, v1q, v2sb, w2sb)

        offs = cst.tile([P, 128], U16, tag="offs", name="offs")
        nc.gpsimd.iota(out=offs[:], pattern=[[1024, 16], [0, 8]], base=0,
                       channel_multiplier=0)
        mskhi = cst.tile([P, 1], U32, tag="mskhi", name="mskhi")
        nc.vector.memset(mskhi[:], 0xFFFFC000)
        msklo = cst.tile([P, 1], U32, tag="msklo", name="msklo")
        nc.vector.memset(msklo[:], 0x3FFF)

        def wrap_idx(src_i16, bounce_ap, tag, bufs=2):
            """src (128, M) i16, flat order i = j*128+p -> replicated wrapped
            idx tile (128, 8*M) via DRAM bounce."""
            M = src_i16.shape[-1]
            bw = bounce_ap.rearrange("c (j e) -> c j e", e=8)
            for ph in range(8):
                nc.sync.dma_start(out=bw[:, 0:M, ph],
                                  in_=src_i16[ph * 16:(ph + 1) * 16, :])
            idxt = wk.tile([P, 8 * M], I16, tag=tag, name=tag, bufs=bufs)
            for g in range(8):
                nc.sync.dma_start(out=idxt[g * 16:(g + 1) * 16, :],
                                  in_=bounce_ap)
            return idxt

        # persistent cross-stage tensors
        nqb1, nqb2 = [], []
        for i in range(2):
            t = cst.tile([4, 512], BF16, tag=f"nqb1{i}", name=f"nqb1{i}")
            nc.vector.memset(t[:], 1.0)
            nqb1.append(t)
            t = cst.tile([4, 512], BF16, tag=f"nqb2{i}", name=f"nqb2{i}")
            nc.vector.memset(t[:], 1.0)
            nqb2.append(t)
        feat1T = cst.tile([P, S1], F32, tag="feat1T", name="feat1T")
        feat2T = [cst.tile([P, S2], F32, tag=f"feat2T{h}", name=f"feat2T{h}")
                  for h in range(2)]
        f1upT = cst.tile([P, S1], F32, tag="f1upT", name="f1upT")
        gfacc = cst.tile([P, 1], F32, tag="gfacc", name="gfacc")
        zroP = cst.tile([P, 512], F32, tag="zroP", name="zroP")
        nc.vector.memset(zroP[:], 0.0)
        biasH = cst.tile([P, 1], F32, tag="biasH", name="biasH")

        def interp_weights(sqpm_ap, v8_ap, nt, w_out):
            """d2 = |q|^2 - m -> dist -> normalized inv-dist weights -> w_out"""
            d2 = wk.tile([P, nt, 3], F32, tag="ipd2", name="ipd2")
            nc.vector.tensor_tensor(
                out=d2[:], in0=sqpm_ap.unsqueeze(2).to_broadcast([P, nt, 3]),
                in1=v8_ap, op=ALU.subtract)
            nc.scalar.activation(out=d2[:], in_=d2[:], func=AF.Relu)
            nc.scalar.activation(out=d2[:], in_=d2[:], func=AF.Sqrt)
            nc.vector.tensor_scalar_max(d2[:], d2[:], 1e-10)
            nc.vector.reciprocal(out=w_out, in_=d2[:])
            ws = wk.tile([P, nt], F32, tag="ipws", name="ipws")
            nc.vector.tensor_reduce(out=ws[:], in_=w_out, axis=AX.X, op=ALU.add)
            nc.vector.reciprocal(out=ws[:], in_=ws[:])
            nc.vector.tensor_tensor(
                out=w_out, in0=w_out,
                in1=ws[:].unsqueeze(2).to_broadcast([P, nt, 3]), op=ALU.mult)

        # ---------------- stage emitters ----------------
        def stage01():
            """embed MLP + tab0 build (PE/ACT/DMA; no DVE)."""
            for q4 in range(4):
                ts_ = slice(q4 * (NT // 4), (q4 + 1) * (NT // 4))
                nc.sync.dma_start(
                    out=tab0_d.rearrange("(t p) c -> p t c", p=P)[
                        :, ts_, 64:68],
                    in_=xyzb[:].rearrange("p (t c) -> p t c", c=4)[:, ts_, :])
            for g in range(16):
                stage = wk.tile([P, 8, 64], BF16, tag="tab0stage",
                                name="tab0stage")
                f0cs = []
                for cc in range(2):
                    c = g * 2 + cc
                    xc = wk.tile([6, 512], F32, tag="xc", name="xc", bufs=2)
                    nc.scalar.dma_start(out=rr(xc[:]),
                                        in_=rr(xT_in[:, c * 512:(c + 1) * 512]))
                    pe = mmtile()
                    mmr(out=pe[:64, :], lhsT=embw[:], rhs=xc[:],
                        start=True, stop=True)
                    f0c = wk.tile([64, 512], BF16, tag="f0c", name="f0c",
                                  bufs=2)
                    nc.scalar.activation(out=f0c[:], in_=pe[:64, :],
                                         func=AF.Relu, bias=embb[:])
                    nc.scalar.dma_start(out=f0T_d[:, c * 512:(c + 1) * 512],
                                          in_=f0c[:])
                    f0cs.append(f0c)
                pt = psb()
                for cc in range(2):
                    for t4 in range(4):
                        blk = cc * 4 + t4
                        nc.tensor.matmul(
                            out=pt[0:128, blk * 64:(blk + 1) * 64],
                            lhsT=f0cs[cc][:, t4 * 128:(t4 + 1) * 128],
                            rhs=identb[0:64, 0:64],
                            is_transpose=True, start=True, stop=True)
                nc.scalar.activation(
                    out=stage[:],
                    in_=pt[0:128, 0:512].rearrange("p (t c) -> p t c", c=64),
                    func=AF.Copy)
                nc.sync.dma_start(
                    out=tab0_d.rearrange("(t p) c -> p t c", p=P)[
                        :, g * 8:(g + 1) * 8, 0:64],
                    in_=stage[:])

        def knn_sel(lhsT_ap, rhs_ap, ncand, nwin, tag):
            """top-32 of ncand candidates for 128 queries: ACT copies each
            PSUM window to SBUF bf16, window top-8 on DVE max8/max_index in
            bf16 (2x scan rate), then 4-round refine + index extraction.
            Returns (P, 32) i16 global candidate indices."""
            ncw = ncand // nwin  # window width
            candV = wk.tile([P, 8 * nwin], F32, tag=f"cV{tag}", name=f"cV{tag}")
            candI = wk.tile([P, 8 * nwin], U16, tag=f"cI{tag}", name=f"cI{tag}")
            for c in range(nwin):
                pm = psS.tile([P, 1024], F32, tag="sel", name="sel")
                for hh in range(ncw // 512):
                    nc.tensor.matmul(
                        out=pm[:, hh * 512:(hh + 1) * 512], lhsT=lhsT_ap,
                        rhs=rhs_ap[:, c * ncw + hh * 512:
                                   c * ncw + (hh + 1) * 512],
                        start=True, stop=True)
                nc.vector.max(out=candV[:, c * 8:(c + 1) * 8],
                              in_=pm[:, 0:ncw])
                nc.vector.max_index(out=candI[:, c * 8:(c + 1) * 8],
                                    in_max=candV[:, c * 8:(c + 1) * 8],
                                    in_values=pm[:, 0:ncw])
            nc.vector.tensor_tensor(out=candI[:], in0=candI[:],
                                    in1=offs[:, 0:8 * nwin], op=ALU.add)
            return knn_refine(candV, candI, nwin, tag)

        def knn_refine(candV, candI, nwin, tag):
            """top-32 of the 8*nwin candidates -> (P,32) i16 indices.
            Packs the global index into the low 14 mantissa bits of each
            candidate value (ties within 2^-9 relative resolve by index -
            below the bf16 coordinate noise), so one 4-round max8 +
            match_replace pass yields the indices directly."""
            candIw = wk.tile([P, 8 * nwin], F32, tag=f"cIf{tag}",
                             name=f"cIf{tag}")
            nc.vector.tensor_copy(out=candIw[:].bitcast(U32), in_=candI[:])
            candP = wk.tile([P, 8 * nwin], F32, tag=f"cVw{tag}",
                            name=f"cVw{tag}")
            nc.vector.tensor_scalar(out=candP[:].bitcast(U32),
                                    in0=candV[:].bitcast(U32),
                                    scalar1=mskhi[:, 0:1], scalar2=None,
                                    op0=ALU.bitwise_and)
            nc.vector.tensor_tensor(out=candP[:].bitcast(U32),
                                    in0=candP[:].bitcast(U32),
                                    in1=candIw[:].bitcast(U32),
                                    op=ALU.bitwise_or)
            selP = wk.tile([P, K1], F32, tag=f"sV{tag}", name=f"sV{tag}")
            for rn in range(4):
                rs = slice(rn * 8, (rn + 1) * 8)
                nc.vector.max(out=selP[:, rs], in_=candP[:])
                if rn < 3:
                    nc.vector.match_replace(out=candP[:],
                                            in_to_replace=selP[:, rs],
                                            in_values=candP[:],
                                            imm_value=NEG)
            selI = wk.tile([P, K1], F32, tag=f"sI{tag}", name=f"sI{tag}")
            nc.vector.tensor_scalar(out=selI[:].bitcast(U32),
                                    in0=selP[:].bitcast(U32),
                                    scalar1=msklo[:, 0:1], scalar2=None,
                                    op0=ALU.bitwise_and)
            nbr16 = wk.tile([P, K1], I16, tag=f"nb{tag}", name=f"nb{tag}")
            nc.vector.tensor_copy(out=nbr16[:], in_=selI[:].bitcast(U32))
            return nbr16

        # ---- SA1 ----
        def sa1_sel(qt):
            nbr16 = knn_sel(q1t[:, qt * P:(qt + 1) * P], bigT[:], N, 16, "s1")
            return wrap_idx(nbr16[:], ib1_ds[qt][:, :], "idxt1", bufs=3)

        def sa1_gather(idxt):
            halves = []
            for hh in range(2):
                g = wk.tile([P, K1 // 2, TAB0_W], BF16, tag="gn", name="gn")
                for k in range(2):
                    kk = hh * 2 + k
                    nc.gpsimd.dma_gather(g[:, k * 8:(k + 1) * 8, :], tab0_d[:],
                                         idxt[:, kk * 64:(kk + 1) * 64],
                                         1024, 1024, TAB0_W)
                halves.append(g)
            return halves

        def sa1_mlp(qt, gnh):
            nq = nqb1[qt % 2]
            nc.scalar.activation(
                out=nq[0:3, :].rearrange("r (j q) -> r j q", q=P),
                in_=nq1sb[0:3, qt * P:(qt + 1) * P].unsqueeze(1).to_broadcast(
                    [3, 4, P]),
                func=AF.Copy)
            acc = wk.tile([P, P], BF16, tag="sa1acc", name="sa1acc")
            for c in range(8):
                gn = gnh[c // 4]
                cl = c % 4
                pg = psb()
                for j in range(4):
                    nc.tensor.matmul(out=pg[0:68, j * 128:(j + 1) * 128],
                                     lhsT=gn[:, cl * 4 + j, 0:68],
                                     rhs=identb[:], is_transpose=True,
                                     start=True, stop=True)
                gt = wk.tile([68, 512], BF16, tag="gt", name="gt", bufs=2)
                nc.scalar.activation(out=gt[:], in_=pg[0:68, :], func=AF.Copy)
                pz = mmtile()
                nc.tensor.matmul(out=pz[:], lhsT=w1aug[:], rhs=gt[:],
                                 start=True, stop=False)
                nc.tensor.matmul(out=pz[:], lhsT=w1q[:], rhs=nq[:],
                                 start=False, stop=True)
                h1 = wk.tile([P, 512], BF16, tag="h1", name="h1", bufs=2)
                nc.scalar.activation(out=h1[:], in_=pz[:], func=AF.Relu)
                pz2 = mmtile()
                nc.tensor.matmul(out=pz2[:], lhsT=w2sb[:], rhs=h1[:],
                                 start=True, stop=True)
                red = wk.tile([P, P], BF16, tag="sa1red", name="sa1red",
                              bufs=2)
                nc.vector.tensor_reduce(
                    out=red[:], in_=pz2[:].rearrange("f (s q) -> f q s", q=P),
                    axis=AX.X, op=ALU.max)
                if c == 0:
                    nc.vector.tensor_copy(out=acc[:], in_=red[:])
                else:
                    nc.vector.tensor_tensor(out=acc[:], in0=acc[:],
                                            in1=red[:], op=ALU.max)
            nc.scalar.activation(out=rr(feat1T[:, qt * P:(qt + 1) * P]),
                                 in_=acc[:], func=AF.Relu, bias=b2sb[:])

        def tab1_write():
            stage1 = wk.tile([P, T1, TAB1_W], BF16, tag="stage1", name="stage1",
                             bufs=1)
            for t in range(T1):
                pf = transf(feat1T[:, t * P:(t + 1) * P])
                nc.scalar.activation(out=stage1[:, t, 0:128], in_=pf,
                                     func=AF.Copy)
            nc.vector.tensor_copy(
                out=stage1[:, :, 128:131],
                in_=x1pm[:].rearrange("p (t c) -> p t c", c=3))
            nc.vector.memset(stage1[:, :, 131:TAB1_W], 0.0)
            for q4 in range(4):
                ts_ = slice(q4 * 2, (q4 + 1) * 2)
                nc.sync.dma_start(
                    out=tab1_d.rearrange("(t p) c -> p t c", p=P)[:, ts_, :],
                    in_=stage1[:, ts_, :])

        # ---- SA2 ----
        def sa2_sel(t2):
            pm2 = psS.tile([P, 1024], F32, tag="sel", name="sel")
            for hh in range(2):
                nc.tensor.matmul(out=pm2[:, hh * 512:(hh + 1) * 512],
                                 lhsT=q2t[:, t2 * P:(t2 + 1) * P],
                                 rhs=c1t[:, hh * 512:(hh + 1) * 512],
                                 start=True, stop=True)
            selV2 = wk.tile([P, K2], F32, tag="selV2", name="selV2")
            selI2 = wk.tile([P, K2], U16, tag="selI2", name="selI2")
            for rn in range(4):
                rs = slice(rn * 8, (rn + 1) * 8)
                nc.vector.max(out=selV2[:, rs], in_=pm2[:])
                nc.vector.max_index(out=selI2[:, rs], in_max=selV2[:, rs],
                                    in_values=pm2[:])
                if rn < 3:
                    nc.vector.match_replace(out=pm2[:],
                                            in_to_replace=selV2[:, rs],
                                            in_values=pm2[:], imm_value=NEG)
            nbr2 = wk.tile([P, K2], I16, tag="nbr2", name="nbr2")
            nc.vector.tensor_copy(out=nbr2[:], in_=selI2[:])
            return wrap_idx(nbr2[:], ib2_ds[t2][:, :], "idxt2")

        def sa2_gather(idxt2):
            halves = []
            for hh in range(2):
                g = wk.tile([P, K2 // 2, TAB1_W], BF16, tag="gn2",
                            name="gn2")
                for k in range(2):
                    kk = hh * 2 + k
                    nc.gpsimd.dma_gather(g[:, k * 8:(k + 1) * 8, :], tab1_d[:],
                                         idxt2[:, kk * 64:(kk + 1) * 64],
                                         1024, 1024, TAB1_W)
                halves.append(g)
            return halves

        def sa2_mlp(t2, gn2h):
            nq2 = nqb2[t2 % 2]
            nc.scalar.activation(
                out=nq2[0:3, :].rearrange("r (j q) -> r j q", q=P),
                in_=nq2sb[0:3, t2 * P:(t2 + 1) * P].unsqueeze(1).to_broadcast(
                    [3, 4, P]),
                func=AF.Copy)
            acc2 = [wk.tile([P, P], BF16, tag=f"sa2acc{h}", name=f"sa2acc{h}")
                    for h in range(2)]
            for c in range(8):
                gn2 = gn2h[c // 4]
                cl = c % 4
                pga = psb()
                pgb = psb()
                for j in range(4):
                    nc.tensor.matmul(out=pga[:, j * 128:(j + 1) * 128],
                                     lhsT=gn2[:, cl * 4 + j, 0:128],
                                     rhs=identb[:], is_transpose=True,
                                     start=True, stop=True)
                    nc.tensor.matmul(out=pgb[0:3, j * 128:(j + 1) * 128],
                                     lhsT=gn2[:, cl * 4 + j, 128:131],
                                     rhs=identb[:], is_transpose=True,
                                     start=True, stop=True)
                gta = wk.tile([P, 512], BF16, tag="gta", name="gta", bufs=2)
                gtb = wk.tile([3, 512], BF16, tag="gtb", name="gtb", bufs=2)
                nc.scalar.activation(out=gta[:], in_=pga[:], func=AF.Copy)
                nc.scalar.activation(out=gtb[:], in_=pgb[0:3, :], func=AF.Copy)
                h1c = []
                for h in range(2):
                    pz = mmtile()
                    nc.tensor.matmul(out=pz[:], lhsT=v1A[h][:], rhs=gta[:],
                                     start=True, stop=False)
                    nc.tensor.matmul(out=pz[:], lhsT=v1rel[h][:], rhs=gtb[:],
                                     start=False, stop=False)
                    nc.tensor.matmul(out=pz[:], lhsT=v1q[h][:], rhs=nq2[:],
                                     start=False, stop=True)
                    hh_ = wk.tile([P, 512], BF16, tag=f"h1c{h}", name=f"h1c{h}",
                                  bufs=2)
                    nc.scalar.activation(out=hh_[:], in_=pz[:], func=AF.Relu)
                    h1c.append(hh_)
                for h in range(2):
                    pz = mmtile()
                    nc.tensor.matmul(out=pz[:], lhsT=v2sb[0][h][:],
                                     rhs=h1c[0][:], start=True, stop=False)
                    nc.tensor.matmul(out=pz[:], lhsT=v2sb[1][h][:],
                                     rhs=h1c[1][:], start=False, stop=True)
                    red = wk.tile([P, P], BF16, tag="sa2red", name="sa2red",
                                  bufs=2)
                    nc.vector.tensor_reduce(
                        out=red[:],
                        in_=pz[:].rearrange("f (s q) -> f q s", q=P),
                        axis=AX.X, op=ALU.max)
                    if c == 0:
                        nc.vector.tensor_copy(out=acc2[h][:], in_=red[:])
                    else:
                        nc.vector.tensor_tensor(out=acc2[h][:], in0=acc2[h][:],
                                                in1=red[:], op=ALU.max)
            for h in range(2):
                nc.scalar.activation(out=feat2T[h][:, t2 * P:(t2 + 1) * P],
                                     in_=acc2[h][:], func=AF.Relu,
                                     bias=c2sb[:, h:h + 1])

        def tab2_write():
            stage2 = wk.tile([P, T2, 256], BF16, tag="stage2", name="stage2",
                             bufs=1)
            for t2 in range(T2):
                for h in range(2):
                    pf = transf(feat2T[h][:, t2 * P:(t2 + 1) * P])
                    nc.scalar.activation(
                        out=stage2[:, t2, h * 128:(h + 1) * 128],
                        in_=pf, func=AF.Copy)
            for q2 in range(2):
                nc.sync.dma_start(
                    out=tab2_d.rearrange("(t p) c -> p t c", p=P)[
                        :, q2:q2 + 1, :],
                    in_=stage2[:, q2:q2 + 1, :])

        # ---- FP2 ----
        def fp2_sel():
            v8f = wk.tile([P, T1, 8], F32, tag="v8f", name="v8f", bufs=1)
            p8f = wk.tile([P, T1, 8], U16, tag="p8f", name="p8f", bufs=1)
            for qt in range(T1):
                pm3 = psS.tile([P, 1024], F32, tag="sel", name="sel")
                nc.tensor.matmul(out=pm3[:, 0:S2],
                                 lhsT=c1t[:, qt * P:(qt + 1) * P],
                                 rhs=rF2[:], start=True, stop=True)
                nc.vector.max(out=v8f[:, qt, :], in_=pm3[:, 0:S2])
                nc.vector.max_index(out=p8f[:, qt, :], in_max=v8f[:, qt, :],
                                    in_values=pm3[:, 0:S2])
            wn2 = wk.tile([P, T1, 3], F32, tag="wn2", name="wn2", bufs=1)
            interp_weights(sq1pm[:], v8f[:, :, 0:3], T1, wn2[:])
            p3f = wk.tile([P, T1 * 3], I16, tag="p3f", name="p3f", bufs=1)
            nc.vector.tensor_copy(
                out=p3f[:].rearrange("p (t j) -> p t j", j=3),
                in_=p8f[:, :, 0:3])
            idxtf2 = wrap_idx(p3f[:], ibf2_d[:, :], "idxtf2")
            return idxtf2, wn2

        def fp2_gather(idxtf2):
            halves = []
            for hh in range(2):
                gi2 = wk.tile([P, T1 // 2, 3, 256], BF16, tag="gi2",
                              name="gi2")
                gi2v = gi2[:].rearrange("p t j c -> p (t j) c")
                # 24 (t,j) row-slots split as 12+12 across the two halves
                for k in range(3):
                    nc.gpsimd.dma_gather(
                        gi2v[:, k * 4:(k + 1) * 4, :], tab2_d[:],
                        idxtf2[:, hh * 96 + k * 32:hh * 96 + (k + 1) * 32],
                        512, 512, 256)
                halves.append(gi2)
            return halves

        def fp2_mlp(gi2h, wn2):
            it2pm = wk.tile([P, T1, 256], BF16, tag="it2pm", name="it2pm",
                            bufs=1)
            tmp2 = wk.tile([P, T1 // 2, 256], BF16, tag="it2tmp",
                           name="it2tmp", bufs=1)
            for hh in range(2):
                gi2 = gi2h[hh]
                ts_ = slice(hh * (T1 // 2), (hh + 1) * (T1 // 2))
                def wb2(j):
                    return wn2[:, ts_, j:j + 1].to_broadcast(
                        [P, T1 // 2, 256])
                nc.vector.tensor_tensor(out=it2pm[:, ts_, :],
                                        in0=gi2[:, :, 1, :], in1=wb2(1),
                                        op=ALU.mult)
                nc.vector.tensor_tensor(out=tmp2[:], in0=gi2[:, :, 2, :],
                                        in1=wb2(2), op=ALU.mult)
                nc.vector.tensor_tensor(out=it2pm[:, ts_, :],
                                        in0=it2pm[:, ts_, :], in1=tmp2[:],
                                        op=ALU.add)
                nc.vector.tensor_tensor(out=tmp2[:], in0=gi2[:, :, 0, :],
                                        in1=wb2(0), op=ALU.mult)
                nc.vector.tensor_tensor(out=it2pm[:, ts_, :],
                                        in0=it2pm[:, ts_, :], in1=tmp2[:],
                                        op=ALU.add)
            itT2 = [wk.tile([P, S1], BF16, tag=f"itT2{h}", name=f"itT2{h}",
                            bufs=1) for h in range(2)]
            for h in range(2):
                for h4 in range(2):
                    pf = psb()
                    for tl in range(4):
                        t = h4 * 4 + tl
                        nc.tensor.matmul(
                            out=pf[0:128, tl * 128:(tl + 1) * 128],
                            lhsT=it2pm[:, t, h * 128:(h + 1) * 128],
                            rhs=identb[:], is_transpose=True,
                            start=True, stop=True)
                    nc.scalar.activation(out=itT2[h][:, h4 * 512:(h4 + 1) * 512],
                                         in_=pf[0:128, 0:512], func=AF.Copy)
            for c in range(2):
                cs = slice(c * 512, (c + 1) * 512)
                pz = mmtile()
                mmr(out=pz[:], lhsT=f2w1[0][:], rhs=feat1T[:, cs],
                    start=True, stop=False)
                nc.tensor.matmul(out=pz[:], lhsT=f2w1[1][:],
                                 rhs=itT2[0][:, cs], start=False, stop=False)
                nc.tensor.matmul(out=pz[:], lhsT=f2w1[2][:],
                                 rhs=itT2[1][:, cs], start=False, stop=True)
                hf = wk.tile([P, 512], F32, tag="fp2h", name="fp2h", bufs=1)
                nc.scalar.activation(out=rr(hf[:]), in_=pz[:], func=AF.Relu,
                                     bias=f2b1[:])
                pz2 = mmtile()
                mmr(out=pz2[:], lhsT=f2w2[:], rhs=hf[:], start=True, stop=True)
                nc.scalar.activation(out=f1upT[:, cs], in_=pz2[:], func=AF.Relu,
                                     bias=f2b2[:])

        def tabf_write():
            stagef = wk.tile([P, T1, 128], BF16, tag="stagef", name="stagef",
                             bufs=1)
            for t in range(T1):
                pf = transf(f1upT[:, t * P:(t + 1) * P])
                nc.scalar.activation(out=stagef[:, t, :], in_=pf, func=AF.Copy)
            for q4 in range(4):
                ts_ = slice(q4 * 2, (q4 + 1) * 2)
                nc.sync.dma_start(
                    out=tabf_d.rearrange("(t p) c -> p t c", p=P)[:, ts_, :],
                    in_=stagef[:, ts_, :])

        # ---- FP1 ----
        def fp1_sel(g):
            v8g = wk.tile([P, GT, 8], F32, tag="v8g", name="v8g", bufs=3)
            p8g = wk.tile([P, GT, 8], U16, tag="p8g", name="p8g", bufs=3)
            for t in range(GT):
                qt = g * GT + t
                pm4 = psS.tile([P, 1024], F32, tag="sel", name="sel")
                for hh in range(2):
                    nc.tensor.matmul(out=pm4[:, hh * 512:(hh + 1) * 512],
                                     lhsT=bigT[:, qt * P:(qt + 1) * P],
                                     rhs=r2x[:, hh * 512:(hh + 1) * 512],
                                     start=True, stop=True)
                nc.vector.max(out=v8g[:, t, :], in_=pm4[:])
                nc.vector.max_index(out=p8g[:, t, :], in_max=v8g[:, t, :],
                                    in_values=pm4[:])
            wng = wk.tile([P, GT, 3], F32, tag="wng", name="wng", bufs=9)
            interp_weights(sqpm[:, g * GT:(g + 1) * GT], v8g[:, :, 0:3], GT,
                           wng[:])
            p3g = wk.tile([P, GT * 3], I16, tag="p3g", name="p3g", bufs=4)
            nc.vector.tensor_copy(
                out=p3g[:].rearrange("p (t j) -> p t j", j=3),
                in_=p8g[:, :, 0:3])
            idxtg = wrap_idx(p3g[:], ibf1_d[:, g * 192:(g + 1) * 192],
                             "idxtf1", bufs=7)
            return idxtg, wng

        def fp1_gather(g, idxtg):
            gi1 = wk.tile([P, GT, 3, 128], BF16, tag="gi1", name="gi1",
                          bufs=3)
            gi1v = gi1[:].rearrange("p t j c -> p (t j) c")
            for k in range(3):
                nc.gpsimd.dma_gather(gi1v[:, k * 8:(k + 1) * 8, :], tabf_d[:],
                                     idxtg[:, k * 64:(k + 1) * 64],
                                     1024, 1024, 128)
            f0Tc = wk.tile([64, GT * 128], BF16, tag="f0Tc", name="f0Tc",
                           bufs=2)
            nc.sync.dma_start(out=f0Tc[:],
                              in_=f0T_d[:, g * GT * P:(g + 1) * GT * P])
            return gi1, f0Tc

        def fp1_mlp(g, gif, wng):
            gi1, f0Tc = gif
            it1pm = wk.tile([P, GT, 128], BF16, tag="it1pm", name="it1pm",
                            bufs=1)
            tmp1 = wk.tile([P, GT, 128], BF16, tag="ittmp", name="ittmp",
                           bufs=1)
            def wb(j):
                return wng[:, :, j:j + 1].to_broadcast([P, GT, 128])
            nc.vector.tensor_tensor(out=it1pm[:], in0=gi1[:, :, 1, :],
                                    in1=wb(1), op=ALU.mult)
            nc.vector.tensor_tensor(out=tmp1[:], in0=gi1[:, :, 2, :],
                                    in1=wb(2), op=ALU.mult)
            nc.vector.tensor_tensor(out=it1pm[:], in0=it1pm[:], in1=tmp1[:],
                                    op=ALU.add)
            nc.vector.tensor_tensor(out=tmp1[:], in0=gi1[:, :, 0, :],
                                    in1=wb(0), op=ALU.mult)
            nc.vector.tensor_tensor(out=it1pm[:], in0=it1pm[:], in1=tmp1[:],
                                    op=ALU.add)
            itT1 = wk.tile([P, GT * 128], BF16, tag="itT1", name="itT1",
                           bufs=1)
            for h4 in range(2):
                pf = psb()
                for tl in range(4):
                    t = h4 * 4 + tl
                    nc.tensor.matmul(out=pf[0:128, tl * 128:(tl + 1) * 128],
                                     lhsT=it1pm[:, t, :],
                                     rhs=identb[:], is_transpose=True,
                                     start=True, stop=True)
                nc.scalar.activation(out=itT1[:, h4 * 512:(h4 + 1) * 512],
                                     in_=pf[0:128, 0:512], func=AF.Copy)
            fus = []
            for c in range(2):
                cs = slice(c * 512, (c + 1) * 512)
                gcs = slice(g * GT * P + c * 512, g * GT * P + (c + 1) * 512)
                pz = mmtile()
                nc.tensor.matmul(out=pz[:], lhsT=f1w1b[:], rhs=itT1[:, cs],
                                 start=True, stop=False)
                nc.tensor.matmul(out=pz[:], lhsT=f1w1a[:], rhs=f0Tc[:, cs],
                                 start=False, stop=True)
                hf = wk.tile([P, 512], F32, tag="fp1h", name="fp1h", bufs=2)
                nc.scalar.activation(out=rr(hf[:]), in_=pz[:], func=AF.Relu,
                                     bias=f1b1[:])
                pz2 = mmtile()
                mmr(out=pz2[:], lhsT=f1w2[:], rhs=hf[:], start=True, stop=True)
                fu = wk.tile([P, 512], BF16, tag="fuc", name="fuc", bufs=3)
                nc.scalar.activation(out=fu[:], in_=pz2[:], func=AF.Relu,
                                     bias=f1b2[:])
                nc.sync.dma_start(out=fuT_d[:, gcs], in_=fu[:])
                fus.append(fu)
            return fus

        def fp1_tail(g, fus):
            """global-max accumulation (DVE) - emitted after sel(g) so it
            never head-of-line blocks the selection scans."""
            for c, fu in enumerate(fus):
                if g == 0 and c == 0:
                    nc.vector.tensor_reduce(out=gfacc[:], in_=fu[:],
                                            axis=AX.X, op=ALU.max)
                else:
                    red1 = wk.tile([P, 1], F32, tag="gfred1", name="gfred1",
                                   bufs=2)
                    nc.vector.tensor_reduce(out=red1[:], in_=fu[:],
                                            axis=AX.X, op=ALU.max)
                    nc.vector.tensor_tensor(out=gfacc[:], in0=gfacc[:],
                                            in1=red1[:], op=ALU.max)

        # ---- head ----
        def head():
            pc = mmtile()
            nc.tensor.matmul(out=pc[:, 0:1], lhsT=hw1b[:], rhs=gfacc[:],
                             start=True, stop=True)
            nc.vector.tensor_tensor(out=biasH[:], in0=pc[:, 0:1], in1=hb1[:],
                                    op=ALU.add)
            for g in range(8):
                for c4 in range(4):
                    c = g * 4 + c4
                    cs = slice(c * 512, (c + 1) * 512)
                    fuc = wk.tile([P, 512], BF16, tag="hfuc", name="hfuc",
                                  bufs=2)
                    nc.sync.dma_start(out=fuc[:], in_=fuT_d[:, cs])
                    pzs = psS.tile([P, 1024], F32, tag="sel", name="sel")
                    pz = pzs[:, 0:512]
                    nc.tensor.matmul(out=pz, lhsT=hw1a[:], rhs=fuc[:],
                                     start=True, stop=True)
                    h1 = wk.tile([P, 512], F32, tag="hh1", name="hh1", bufs=2)
                    if c % 2 == 0:
                        nc.scalar.activation(out=rr(h1[:]), in_=pz,
                                             func=AF.Relu, bias=biasH[:])
                    else:
                        nc.vector.scalar_tensor_tensor(
                            out=rr(h1[:]), in0=pz, scalar=biasH[:],
                            in1=zroP[:], op0=ALU.add, op1=ALU.max)
                    pz2 = mmtile()
                    mmr(out=pz2[:64, :], lhsT=hw2[:], rhs=h1[:],
                        start=True, stop=True)
                    h2 = wk.tile([64, 512], F32, tag="hh2", name="hh2", bufs=2)
                    nc.vector.scalar_tensor_tensor(
                        out=rr(h2[:]), in0=pz2[:64, :], scalar=hb2[:],
                        in1=zroP[0:64, :], op0=ALU.add, op1=ALU.max)
                    pz3t = psT.tile([P, 512], F32, tag="trans", name="trans")
                    pz3 = pz3t[:]
                    mmr(out=pz3[:13, :], lhsT=hw3[:], rhs=h2[:],
                        start=True, stop=True)
                    oT = wk.tile([13, 512], F32, tag="hoT", name="hoT", bufs=2)
                    nc.scalar.activation(out=oT[:], in_=pz3[:13, :],
                                         func=AF.Identity, bias=hb3[:])
                    nc.sync.dma_start(
                        out=out_d[:, c * 512:(c + 1) * 512], in_=oT[:])

        # ---------------- global software pipeline ----------------
        # DVE runs the selection stream back-to-back; PE/ACT/Pool/DMA run the
        # embed->SA1->SA2->FP2->FP1->head MLP chain in the gaps.
        idx1 = {}
        idx1[0] = sa1_sel(0)
        stage01()
        idx1[1] = sa1_sel(1)
        idx1[2] = sa1_sel(2)
        (c1t, r2x, q2t, rF2, nq1sb, nq2sb, sqpm, sq1pm, x1pm) = late_loads()
        (b2sb, c2sb, f2b1, f2w2, f2b2, f1b1, f1w2, f1b2, hb1, hw2, hb2, hw3,
         hb3, f2w1, f1w1a, f1w1b, hw1a, hw1b, w1aug, w1q, v1A, v1rel, v1q,
         v2sb, w2sb) = late_weights()
        gn1 = {}
        gn1[0] = sa1_gather(idx1[0])
        gn1[1] = sa1_gather(idx1[1])
        for qt in range(3, T1):
            idx1[qt] = sa1_sel(qt)
            gn1[qt - 1] = sa1_gather(idx1[qt - 1])
            sa1_mlp(qt - 3, gn1[qt - 3])
        idx2 = {}
        idx2[0] = sa2_sel(0)
        gn1[T1 - 1] = sa1_gather(idx1[T1 - 1])
        sa1_mlp(T1 - 3, gn1[T1 - 3])
        idx2[1] = sa2_sel(1)
        sa1_mlp(T1 - 2, gn1[T1 - 2])
        idxf2, wn2 = fp2_sel()
        sa1_mlp(T1 - 1, gn1[T1 - 1])
        tab1_write()
        idxg = {}
        wng = {}
        g2 = {}
        g2[0] = sa2_gather(idx2[0])
        idxg[0], wng[0] = fp1_sel(0)
        g2[1] = sa2_gather(idx2[1])
        sa2_mlp(0, g2[0])
        idxg[1], wng[1] = fp1_sel(1)
        sa2_mlp(1, g2[1])
        tab2_write()
        gi2h = fp2_gather(idxf2)
        idxg[2], wng[2] = fp1_sel(2)
        fp2_mlp(gi2h, wn2)
        tabf_write()
        gi = {}
        gi[0] = fp1_gather(0, idxg[0])
        gi[1] = fp1_gather(1, idxg[1])
        for g in range(3, NG):
            idxg[g], wng[g] = fp1_sel(g)
            gi[g - 1] = fp1_gather(g - 1, idxg[g - 1])
            fp1_tail(g - 3, fp1_mlp(g - 3, gi[g - 3], wng[g - 3]))
        gi[NG - 1] = fp1_gather(NG - 1, idxg[NG - 1])
        fp1_tail(NG - 3, fp1_mlp(NG - 3, gi[NG - 3], wng[NG - 3]))
        fp1_tail(NG - 2, fp1_mlp(NG - 2, gi[NG - 2], wng[NG - 2]))
        fp1_tail(NG - 1, fp1_mlp(NG - 1, gi[NG - 1], wng[NG - 1]))
        head()

    return nc


# ---------------------------------------------------------------- host side
_CACHED_NC = None


def _get_nc():
    global _CACHED_NC
    if _CACHED_NC is None:
        nc = build_nc()
        nc.finalize()
        _CACHED_NC = nc
    return _CACHED_NC


def _per_core_inputs(b, inputs):
    import ml_dtypes
    bf16 = ml_dtypes.bfloat16
    x = np.ascontiguousarray(np.asarray(inputs["x"][b]), dtype=np.float32)
    i1 = np.asarray(inputs["idx_s1"][b]).astype(np.int64)
    i2 = np.asarray(inputs["idx_s2"][b]).astype(np.int64)
    xyz = x[:, 0:3].astype(np.float32)
    sq = (xyz * xyz).sum(1)                      # |x|^2  (N,)
    x1 = xyz[i1]                                 # (S1,3)
    sq1 = (x1 * x1).sum(1)
    x2 = x1[i2]                                  # (S2,3)
    sq2 = (x2 * x2).sum(1)
    one = np.ones
    zer = np.zeros
    f32 = lambda a: np.ascontiguousarray(np.asarray(a), dtype=np.float32)
    b16 = lambda a: np.ascontiguousarray(np.asarray(a, dtype=np.float32)
                                         .astype(bf16))
    # Scores are shifted by -|q|^2 (the spare row in each table) so the PE
    # emits ~ -d^2 + delta_q: small values that survive the bf16 PSUM->SBUF
    # copy with ~2^-9 relative error.  delta_q = |q|^2 - f32(bf16(|q|^2)) is
    # the query-constant residue; sqpm/sq1pm carry it for the interp weights
    # (d^2 = delta_q - score').
    bigT = np.concatenate([xyz.T, -sq[None], one((1, N))], 0)
    q1t = np.concatenate([2 * x1.T, one((1, S1)), -sq1[None]], 0)
    c1t = np.concatenate([x1.T, -sq1[None], one((1, S1))], 0)
    r2x = np.concatenate([2 * x1.T, one((1, S1)), -sq1[None]], 0)
    q2t = np.concatenate([2 * x2.T, one((1, S2)), -sq2[None]], 0)
    rF2 = np.concatenate([2 * x2.T, one((1, S2)), -sq2[None]], 0)
    dlt = sq - np.asarray(sq.astype(bf16), dtype=np.float32)
    dlt1 = sq1 - np.asarray(sq1.astype(bf16), dtype=np.float32)
    xyzb = np.concatenate([xyz.reshape(NT, P, 3), np.zeros((NT, P, 1))],
                          2).transpose(1, 0, 2).reshape(P, NT * 4)
    w1 = f32(inputs["sa1_w1"])                   # (67,128): [rel(3);feat(64)]
    b1 = f32(inputs["sa1_b1"]).reshape(1, 128)
    w1aug = np.concatenate([w1[3:67], w1[0:3], zer((1, 128))], 0)
    w1q = np.concatenate([w1[0:3], b1], 0)
    v1 = f32(inputs["sa2_w1"])                   # (131,256)
    c1r = f32(inputs["sa2_b1"]).reshape(1, 256)
    v1q = np.concatenate([v1[0:3], c1r], 0)
    return {
        "xT": np.ascontiguousarray(x.T),
        "bigT": b16(bigT),
        "q1t": b16(q1t),
        "c1t": b16(c1t),
        "r2x": b16(r2x),
        "q2t": b16(q2t),
        "rF2": b16(rF2),
        "nq1": b16(-x1.T),
        "nq2": b16(-x2.T),
        "sqpm": f32(dlt.reshape(NT, P).T),
        "sq1pm": f32(dlt1.reshape(T1, P).T),
        "xyzb": b16(xyzb),
        "x1pm": b16(x1.reshape(T1, P, 3).transpose(1, 0, 2).reshape(P, T1 * 3)),
        "w1augb": b16(w1aug),
        "w1qb": b16(w1q),
        "v1Ab": b16(v1[3:131]),
        "v1relb": b16(v1[0:3]),
        "v1qb": b16(v1q),
        "v2b": b16(inputs["sa2_w2"]),
        "w2b": b16(inputs["sa1_w2"]),
        "f2w1b": b16(f32(inputs["fp2_w1"])[128:384]),
        "f1w1ab": b16(f32(inputs["fp1_w1"])[0:64]),
        "f1w1bb": b16(f32(inputs["fp1_w1"])[64:192]),
        "hw1ab": b16(f32(inputs["head_w1"])[0:128]),
        "embw": f32(inputs["embed_w"]),
        "embb": f32(inputs["embed_b"]).reshape(64, 1),
        "b2": f32(inputs["sa1_b2"]).reshape(128, 1),
        "c2": np.ascontiguousarray(f32(inputs["sa2_b2"]).reshape(2, 128).T),
        "f2w1": f32(inputs["fp2_w1"]),
        "f2b1": f32(inputs["fp2_b1"]).reshape(128, 1),
        "f2w2": f32(inputs["fp2_w2"]),
        "f2b2": f32(inputs["fp2_b2"]).reshape(128, 1),
        "f1w1": f32(inputs["fp1_w1"]),
        "f1b1": f32(inputs["fp1_b1"]).reshape(128, 1),
        "f1w2": f32(inputs["fp1_w2"]),
        "f1b2": f32(inputs["fp1_b2"]).reshape(128, 1),
        "hw1": f32(inputs["head_w1"]),
        "hb1": f32(inputs["head_b1"]).reshape(128, 1),
        "hw2": f32(inputs["head_w2"]),
        "hb2": f32(inputs["head_b2"]).reshape(64, 1),
        "hw3": f32(inputs["head_w3"]),
        "hb3": f32(inputs["head_b3"]).reshape(13, 1),
    }


def run(inputs, trace=False, **kw):
    nc = _get_nc()
    B = inputs["x"].shape[0]
    in_maps = [_per_core_inputs(b, inputs) for b in range(B)]
    res = run_bass_kernel_spmd(nc, in_maps, core_ids=list(range(B)),
                               trace=trace, **kw)
    out = np.stack([np.ascontiguousarray(res.results[b]["out"].T)
                    for b in range(B)])
    return out, res


def kernel(**inputs):
    return run(inputs)[0]


if __name__ == "__main__":
    build_nc()
    print("built ok")

